# revision 9
# baseline (speedup 1.0000x reference)
"""CCMLite kernel for Trainium2: GroupNorm(affine=False) + low-rank channel mix.

out = x_norm + u @ (v^T @ x_norm) + shift, with x_norm = groupnorm(x).

Sharding: data-parallel over batch B=16 across 8 cores (2 batch elems/core).

Pipeline-first redesign vs the previous version: the kernel is DMA-bound
(8.4 MB/core at ~360 GB/s ~= 23.5 us floor), so every phase is arranged to
keep the DMA engines streaming continuously:
  - all 8 x-tile loads issue up-front on the sync HWDGE ring
  - stats are split per-tile between DVE (bn_stats / 4x-mode sum-accum) and
    ACT (Square+accum_out), so per-batch stats lag the loads by ~2 us
  - stage A (vtx = vs^T x) packs 4 rank-12 strips per PSUM tile via
    tile_position; one wide DVE copy evacuates each column-half, and a tiny
    DMA restores the "ones" rows used to carry cst through stage B matmuls
  - stage B units of [128,1024] are routed per-unit to one of three paths:
      D: DVE scalar_tensor_tensor reads PSUM directly   (s*x + (u@vtx+cst))
      A: extra PE diag(s) matmul + ACT Identity+bias     (cheapest per-col)
      P: Pool (gpsimd) scalar_tensor_tensor              (otherwise-idle engine)
    so the combine work is spread across DVE/ACT/Pool under the DMA floor
  - each unit's [128,1024] output store DMAs immediately on the sync ring
"""

from contextlib import ExitStack

import numpy as np

import concourse.bass as bass
import concourse.tile as tile
from concourse import bacc, mybir
from concourse.bass_utils import run_bass_kernel_spmd

N_CORES = 8
B, C, H, W = 16, 256, 64, 64
HW = H * W            # 4096
R = 12                # low rank
G = 32                # groups
GPC = C // G          # 8 channels per group
P = 128               # partitions
CB = C // P           # 2 channel blocks
BPC = B // N_CORES    # 2 batch elements per core
EPS = 1e-6
F32 = mybir.dt.float32
F16 = mybir.dt.float16

_MULT = mybir.AluOpType.mult
_ADD = mybir.AluOpType.add
_SUB = mybir.AluOpType.subtract
AF = mybir.ActivationFunctionType

# ---- schedule knobs ----
# stats method per (b, cb): tuple over the two [128,2048] tiles,
#   'bn'  = DVE bn_stats (both moments, heavier on DVE)
#   'act' = ACT Square+accum for sumsq + DVE 4x-mode copy+accum for sum
DEF_STATS = {
    (0, 0): ("bn", "bn"),
    (0, 1): ("bn", "bn"),
    (1, 0): ("act", "act"),
    (1, 1): ("act", "act"),
}
# stage-B path per batch: 8 chars, unit order (k, cb) k-major.
#   D = DVE STT reads PSUM; A = PE diag + ACT Identity+bias;
#   E = DVE 4x t=s*x, ACT evac to SBUF fp16 (+cst bias), Pool TT-add
DEF_PATHS = ("DEDAEDAE", "DAEDAEAE")
DEF_WARM = 0  # PE warm-up matmuls before stage A


def build_nc(paths=DEF_PATHS, stats=DEF_STATS, warm=DEF_WARM):
    nc = bacc.Bacc(None, target_bir_lowering=False)
    x_d = nc.dram_tensor("x", [BPC, C, HW], F16, kind="ExternalInput")
    aug_d = nc.dram_tensor("aug", [BPC, P, C], F16, kind="ExternalInput")
    vsh_d = nc.dram_tensor("vsh", [BPC, CB, P, 33], F32, kind="ExternalInput")
    gmask_d = nc.dram_tensor("gmask", [P, 16], F32, kind="ExternalInput")
    gmaskT_d = nc.dram_tensor("gmaskT", [16, P], F32, kind="ExternalInput")
    ident_d = nc.dram_tensor("ident16", [P, P], F16, kind="ExternalInput")
    ones_d = nc.dram_tensor("ones16", [4, 512], F16, kind="ExternalInput")
    out_d = nc.dram_tensor("out", [BPC, C, HW], F16, kind="ExternalOutput")

    with tile.TileContext(nc) as tc, ExitStack() as ctx:
        consts = ctx.enter_context(tc.tile_pool(name="consts", bufs=1))
        xbp = ctx.enter_context(tc.tile_pool(name="xbp", bufs=8))
        junkp = ctx.enter_context(tc.tile_pool(name="junkp", bufs=2))
        outp = ctx.enter_context(tc.tile_pool(name="outp", bufs=6))
        vtp = ctx.enter_context(tc.tile_pool(name="vtp", bufs=3))
        smalls = ctx.enter_context(tc.tile_pool(name="smalls", bufs=2))
        ps_small = ctx.enter_context(
            tc.tile_pool(name="ps_small", bufs=2, space="PSUM"))
        ps_vtx = ctx.enter_context(tc.tile_pool(name="ps_vtx", bufs=1, space="PSUM"))
        ps_pm = ctx.enter_context(tc.tile_pool(name="ps_pm", bufs=2, space="PSUM"))

        # ---- consts ----
        gmask = consts.tile([P, 16], F32)
        nc.gpsimd.dma_start(out=gmask, in_=gmask_d[:, :])
        gmaskT = consts.tile([16, P], F32)
        nc.gpsimd.dma_start(out=gmaskT, in_=gmaskT_d[:, :])
        ident_h = consts.tile([P, P], F16)
        nc.gpsimd.dma_start(out=ident_h, in_=ident_d[:, :])
        ones4 = consts.tile([4, 512], F16)
        nc.gpsimd.dma_start(out=ones4, in_=ones_d[:, :])
        ones14 = consts.tile([1, 4], F16)
        nc.gpsimd.dma_start(out=ones14, in_=ones_d[0:1, 0:4])
        eps_t = consts.tile([16, 1], F32)
        nc.vector.memset(eps_t, EPS)

        # per-batch params on the scalar HWDGE ring (issued before x loads)
        augs, vshs = [], []
        for b in range(BPC):
            aug = smalls.tile([P, 2 * P], F16, tag=f"aug{b}", bufs=1)
            nc.scalar.dma_start(out=aug, in_=aug_d[b])
            vsh = smalls.tile([P, 66], F32, tag=f"vsh{b}", bufs=1)
            for cb in range(CB):
                nc.scalar.dma_start(
                    out=vsh[:, 33 * cb:33 * (cb + 1)], in_=vsh_d[b, cb])
            augs.append(aug)
            vshs.append(vsh)

        # warm the ACT tables early so Sqrt/Square don't pay a table load on
        # the critical path
        twarm = smalls.tile([16, 1], F32, tag="twarm", bufs=1)
        nc.scalar.activation(out=twarm, in_=eps_t, func=AF.Square)
        nc.scalar.activation(out=twarm, in_=eps_t, func=AF.Sqrt,
                             bias=eps_t[:, 0:1], scale=1.0)
        nc.scalar.activation(out=twarm, in_=eps_t, func=AF.Identity)

        # ---- all x loads up-front, sync ring ----
        xbt = {}
        for b in range(BPC):
            for cb in range(CB):
                for h in range(2):
                    tb = xbp.tile([P, 2048], F16, tag="xbt")
                    nc.sync.dma_start(
                        out=tb,
                        in_=x_d[b, cb * P:(cb + 1) * P, h * 2048:(h + 1) * 2048])
                    xbt[(b, cb, h)] = tb

        if warm:
            wps = ps_small.tile([P, 512], F32, tag="ps")
            for _ in range(warm):
                nc.tensor.matmul(wps[:, 0:P], lhsT=ident_h, rhs=ident_h,
                                 start=True, stop=True)

        # ---- per-(b,cb) stats + small chain ----
        sms = {}    # (b,cb) -> [128,2] f32: col0 rstd, col1 mean
        vss = {}    # (b,cb) -> [128,R] f16 (v*s)
        diags = {}  # (b,cb) -> [128,128] f16 diag(s)
        kvsb = {}   # (b,cb) -> [R,1] f32 partial kvec

        def emit_stats(b, cb):
            acc1 = smalls.tile([P, 2], F32, tag=f"acc1_{b}{cb}", bufs=1)
            acc2 = smalls.tile([P, 2], F32, tag=f"acc2_{b}{cb}", bufs=1)
            for t in range(2):
                xt = xbt[(b, cb, t)]
                if stats[(b, cb)][t] == "bn":
                    st = smalls.tile([P, 4, 6], F32, tag="st")
                    for i in range(4):
                        nc.vector.bn_stats(
                            out=st[:, i:i + 1, :], in_=xt[:, 512 * i:512 * (i + 1)])
                    mv = smalls.tile([P, 2], F32, tag="mv")
                    nc.vector.bn_aggr(out=mv, in_=st)
                    nc.vector.tensor_scalar_mul(
                        out=acc1[:, t:t + 1], in0=mv[:, 0:1], scalar1=2048.0)
                    # sumsq = (m^2 + v) * 2048
                    tmp = smalls.tile([P, 1], F32, tag="tmp")
                    nc.vector.scalar_tensor_tensor(
                        out=tmp, in0=mv[:, 0:1], scalar=mv[:, 0:1],
                        in1=mv[:, 1:2], op0=_MULT, op1=_ADD)
                    nc.vector.tensor_scalar_mul(
                        out=acc2[:, t:t + 1], in0=tmp, scalar1=2048.0)
                else:
                    ja = junkp.tile([P, 2048], F16, tag="ja")
                    nc.scalar.activation(
                        out=ja, in_=xt, func=AF.Square,
                        accum_out=acc2[:, t:t + 1])
                    jd = junkp.tile([P, 2048], F16, tag="jd")
                    nc.vector.tensor_scalar(
                        out=jd, in0=xt, scalar1=1.0, scalar2=0.0,
                        op0=_MULT, op1=_ADD, accum_out=acc1[:, t:t + 1])
            # per-channel mean / E[x^2]
            msum = smalls.tile([P, 2], F32, tag="msum")
            nc.vector.tensor_scalar(
                out=msum[:, 0:1], in0=acc1[:, 0:1], scalar1=acc1[:, 1:2],
                scalar2=1.0 / HW, op0=_ADD, op1=_MULT)
            nc.vector.tensor_scalar(
                out=msum[:, 1:2], in0=acc2[:, 0:1], scalar1=acc2[:, 1:2],
                scalar2=1.0 / HW, op0=_ADD, op1=_MULT)
            # group reduce + broadcast
            gs = ps_small.tile([16, 2], F32, tag="ps")
            nc.tensor.matmul(gs, lhsT=gmask, rhs=msum, start=True, stop=True)
            gvals = smalls.tile([16, 2], F32, tag="gvals")
            tmpg = smalls.tile([16, 4], F32, tag="tmpg")
            nc.vector.tensor_scalar_mul(
                out=gvals[:, 1:2], in0=gs[:, 0:1], scalar1=1.0 / GPC)
            nc.vector.tensor_scalar_mul(
                out=tmpg[:, 0:1], in0=gs[:, 1:2], scalar1=1.0 / GPC)
            nc.vector.tensor_mul(
                out=tmpg[:, 1:2], in0=gvals[:, 1:2], in1=gvals[:, 1:2])
            nc.vector.tensor_sub(
                out=tmpg[:, 2:3], in0=tmpg[:, 0:1], in1=tmpg[:, 1:2])
            nc.scalar.activation(
                out=tmpg[:, 3:4], in_=tmpg[:, 2:3], func=AF.Sqrt,
                bias=eps_t[:, 0:1], scale=1.0)
            nc.vector.reciprocal(out=gvals[:, 0:1], in_=tmpg[:, 3:4])
            bc = ps_small.tile([P, 2], F32, tag="ps")
            nc.tensor.matmul(bc, lhsT=gmaskT, rhs=gvals, start=True, stop=True)
            sm = smalls.tile([P, 2], F32, tag=f"sm{b}{cb}", bufs=1)
            nc.vector.tensor_copy(out=sm, in_=bc)
            sms[(b, cb)] = sm
            # vs = v * s, zero-padded to 32 cols so stage A strips write all
            # 32 partitions (avoids reading uninitialized PSUM in the evac)
            vsf = smalls.tile([P, 32], F32, tag=f"vsf{b}{cb}", bufs=1)
            nc.vector.tensor_scalar_mul(
                out=vsf, in0=vshs[b][:, 33 * cb:33 * cb + 32],
                scalar1=sm[:, 0:1])
            vs = smalls.tile([P, 32], F16, tag=f"vs{b}{cb}", bufs=1)
            nc.vector.tensor_copy(out=vs, in_=vsf)
            vss[(b, cb)] = vs
            if "A" in paths[b]:
                diag = smalls.tile([P, P], F16, tag=f"diag{b}{cb}", bufs=1)
                nc.vector.tensor_scalar_mul(out=diag, in0=ident_h,
                                            scalar1=sm[:, 0:1])
                diags[(b, cb)] = diag
            # kvec partial: kv[r] = sum_c vs[c,r]*m_c
            kv = ps_small.tile([32, 1], F32, tag="ps")
            nc.tensor.matmul(kv, lhsT=vsf, rhs=sm[:, 1:2], start=True, stop=True)
            kvp = smalls.tile([R, 1], F32, tag=f"kv{b}{cb}", bufs=1)
            nc.vector.tensor_copy(out=kvp, in_=kv[0:R, :])
            kvsb[(b, cb)] = kvp

        def emit_cst(b):
            # cst_c = shift_c - m_c*s_c - sum_r u[c,r]*kvec[r]; write into
            # aug row R via transpose + SBUF->SBUF DMA
            kvs = smalls.tile([R, 1], F32, tag=f"kvs{b}", bufs=1)
            nc.vector.tensor_add(out=kvs, in0=kvsb[(b, 0)], in1=kvsb[(b, 1)])
            kvs16 = smalls.tile([R, 1], F16, tag=f"kvs16{b}", bufs=1)
            nc.vector.tensor_copy(out=kvs16, in_=kvs)
            for cb in range(CB):
                aug = augs[b]
                sm = sms[(b, cb)]
                ukv = ps_small.tile([P, 1], F32, tag="ps")
                nc.tensor.matmul(
                    ukv, lhsT=aug[0:R, P * cb:P * (cb + 1)], rhs=kvs16,
                    start=True, stop=True)
                cst = smalls.tile([P, 1], F32, tag=f"cst{b}{cb}", bufs=1)
                nc.vector.tensor_mul(out=cst, in0=sm[:, 1:2], in1=sm[:, 0:1])
                nc.vector.tensor_sub(
                    out=cst,
                    in0=vshs[b][:, 33 * cb + 32:33 * cb + 33],
                    in1=cst)
                nc.vector.tensor_sub(out=cst, in0=cst, in1=ukv)
                csts[(b, cb)] = cst
                cst16 = smalls.tile([P, 1], F16, tag="cst16")
                nc.vector.tensor_copy(out=cst16, in_=cst)
                ctp = ps_small.tile([1, P], F16, tag="ps")
                nc.tensor.transpose(out=ctp, in_=cst16, identity=ident_h)
                cstrow = smalls.tile([1, P], F16, tag="cstrow")
                nc.scalar.copy(out=cstrow, in_=ctp)
                ctp4 = ps_small.tile([4, P], F32, tag="ps")
                nc.tensor.matmul(ctp4, lhsT=ones14, rhs=cstrow,
                                 start=True, stop=True)
                cstrow4 = smalls.tile([4, P], F16, tag="cstrow4")
                nc.scalar.copy(out=cstrow4, in_=ctp4)
                pstride = aug.ap[0][0]
                dst = bass.AP(
                    tensor=aug.tensor,
                    offset=aug.offset + R * pstride + P * cb,
                    ap=[[32 * pstride, 4], [1, P]])
                nc.gpsimd.dma_start(out=dst, in_=cstrow4)

        csts = {}

        def emit_stage_a(b, ch):
            # vtx strips for chunks j = 4*ch + q into ps tile [128,512];
            # strip q occupies partitions 32q..32q+12
            vps = vtx_ps[b]
            for cb in range(CB):
                for q in range(4):
                    nc.tensor.matmul(
                        vps[32 * q:32 * q + 32, 512 * ch:512 * (ch + 1)],
                        lhsT=vss[(b, cb)],
                        rhs=xbt[(b, cb, ch)][:, 512 * q:512 * (q + 1)],
                        start=(cb == 0), stop=(cb == CB - 1),
                        tile_position=(0, 32 * q),
                        skip_group_check=True)

        def emit_evac(b, ch):
            vt = vtp.tile([P, 512], F16, tag="vt")
            nc.vector.tensor_copy(
                out=vt, in_=vtx_ps[b][:, 512 * ch:512 * (ch + 1)])
            # restore ones rows (partitions 12,44,76,108) clobbered by the
            # full-tile copy; engines can't write at partition offset 12, DMA can
            pstride = vt.ap[0][0]
            dst = bass.AP(
                tensor=vt.tensor,
                offset=vt.offset + 12 * pstride,
                ap=[[32 * pstride, 4], [1, 512]])
            nc.gpsimd.dma_start(out=dst, in_=ones4[:, :])
            vts[(b, ch)] = vt

        vts = {}
        vtx_ps = {}

        def emit_unit(b, k, cb):
            # output unit [128,1024]: chunks (2k, 2k+1); x tile h=k//2,
            # cols (k%2)*1024; vtx strips q = 2k%4, (2k+1)%4 in vts[(b, k//2... )]
            path = paths[b][2 * k + cb]
            h, half = k // 2, k % 2
            aug = augs[b]
            sm = sms[(b, cb)]
            x_ap = xbt[(b, cb, h)][:, 1024 * half:1024 * (half + 1)]
            pm = ps_pm.tile([P, 1024], F32, tag="pm")
            for j2 in range(2):
                q = 2 * half + j2
                vt = vts[(b, h)]
                pslice = pm[:, 512 * j2:512 * (j2 + 1)]
                if path == "A":
                    nc.tensor.matmul(
                        pslice, lhsT=diags[(b, cb)],
                        rhs=xbt[(b, cb, h)][:, 512 * q:512 * (q + 1)],
                        start=True, stop=False,
                        skip_group_check=True)
                    nc.tensor.matmul(
                        pslice,
                        lhsT=aug[32 * q:32 * q + R, P * cb:P * (cb + 1)],
                        rhs=vt[32 * q:32 * q + R, :],
                        start=False, stop=True,
                        tile_position=(32 * q, 0),
                        skip_group_check=True)
                else:
                    nc.tensor.matmul(
                        pslice,
                        lhsT=aug[32 * q:32 * q + R + 1, P * cb:P * (cb + 1)],
                        rhs=vt[32 * q:32 * q + R + 1, :],
                        start=True, stop=True,
                        tile_position=(32 * q, 0),
                        skip_group_check=True)
            osb = outp.tile([P, 1024], F16, tag="osb")
            if path == "A":
                nc.scalar.activation(
                    out=osb, in_=pm, func=AF.Identity,
                    bias=csts[(b, cb)], scale=1.0)
            elif path == "D":
                nc.vector.scalar_tensor_tensor(
                    out=osb, in0=x_ap, scalar=sm[:, 0:1], in1=pm,
                    op0=_MULT, op1=_ADD)
            else:  # E: DVE 4x t=s*x, ACT evac (+cst bias), Pool TT-add
                t = outp.tile([P, 1024], F16, tag="tsx", bufs=3)
                nc.vector.tensor_scalar(
                    out=t, in0=x_ap, scalar1=sm[:, 0:1], scalar2=0.0,
                    op0=_MULT, op1=_ADD)
                pmsb = outp.tile([P, 1024], F16, tag="pmsb", bufs=3)
                nc.scalar.activation(out=pmsb, in_=pm, func=AF.Identity)
                nc.gpsimd.tensor_add(out=osb, in0=t, in1=pmsb)
            nc.sync.dma_start(
                out=out_d[b, cb * P:(cb + 1) * P, 1024 * k:1024 * (k + 1)],
                in_=osb)

        # ================= schedule =================
        # batch 0 head
        vtx_ps[0] = ps_vtx.tile([P, 1024], F32, tag="vtx", name="vtx0")
        emit_stats(0, 0)
        emit_stats(0, 1)
        emit_cst(0)
        emit_stage_a(0, 0)
        emit_evac(0, 0)
        emit_stage_a(0, 1)
        # b1 stats for cb0 land during b0 compute; emit before b0 evac/combines
        emit_stats(1, 0)
        emit_evac(0, 1)
        emit_stats(1, 1)
        emit_cst(1)
        # b0 stage B
        for k in range(4):
            for cb in range(CB):
                emit_unit(0, k, cb)
        # b1 tail
        vtx_ps[1] = ps_vtx.tile([P, 1024], F32, tag="vtx", name="vtx1")
        emit_stage_a(1, 0)
        emit_evac(1, 0)
        emit_stage_a(1, 1)
        emit_evac(1, 1)
        for k in range(4):
            for cb in range(CB):
                emit_unit(1, k, cb)

    nc.finalize()
    return nc


def _host_prep(x, ccm_params):
    x = np.asarray(x, dtype=np.float32).reshape(B, C, HW).astype(np.float16)
    x = np.ascontiguousarray(x)
    cp = np.asarray(ccm_params, dtype=np.float32)
    u = cp[:, :C * R].reshape(B, C, R)
    v = cp[:, C * R:2 * C * R].reshape(B, C, R)
    shift = cp[:, 2 * C * R:].reshape(B, C)
    # aug: [B, 128, C] fp16; strips s=0..3: rows 32s..32s+11 = u^T,
    # row 32s+12 = cst written on device
    aug = np.zeros((B, P, C), np.float16)
    ut = u.transpose(0, 2, 1).astype(np.float16)
    for sx in range(4):
        aug[:, 32 * sx:32 * sx + R, :] = ut
    aug = np.ascontiguousarray(aug)
    # vsh: [B, CB, P, 33] f32: cols 0..11 = v, 12..31 zero pad, col 32 = shift
    vsh = np.zeros((B, CB, P, 33), np.float32)
    vsh[..., :R] = v.reshape(B, CB, P, R)
    vsh[..., 32] = shift.reshape(B, CB, P)
    vsh = np.ascontiguousarray(vsh)
    gmask = np.zeros((P, 16), np.float32)
    gmask[np.arange(P), np.arange(P) // GPC] = 1.0
    gmaskT = np.ascontiguousarray(gmask.T)
    ident16 = np.eye(P, dtype=np.float16)
    ones16 = np.ones((4, 512), np.float16)
    in_maps = []
    for c in range(N_CORES):
        bs = slice(c * BPC, (c + 1) * BPC)
        in_maps.append({
            "x": x[bs], "aug": aug[bs], "vsh": vsh[bs],
            "gmask": gmask, "gmaskT": gmaskT, "ident16": ident16,
            "ones16": ones16,
        })
    return in_maps


def kernel(x, ccm_params, _trace=False, _paths=DEF_PATHS, _stats=DEF_STATS,
           _warm=DEF_WARM, **_ignored):
    in_maps = _host_prep(x, ccm_params)
    nc = build_nc(paths=_paths, stats=_stats, warm=_warm)
    res = run_bass_kernel_spmd(
        nc, in_maps, core_ids=list(range(N_CORES)), trace=_trace)
    out = np.concatenate([r["out"] for r in res.results], axis=0)
    out = out.reshape(B, C, H, W).astype(np.float32, copy=False)
    if _trace:
        return out, res
    return out


# revision 11
# speedup vs baseline: 1.0640x; 1.0640x over previous
"""CCMLite kernel for Trainium2: GroupNorm(affine=False) + low-rank channel mix.

out = x_norm + u @ (v^T @ x_norm) + shift, with x_norm = groupnorm(x).

Sharding: data-parallel over batch B=16 across 8 cores (2 batch elems/core).

The kernel is DMA-bound (8.4 MB/core at ~360 GB/s ~= 23.5 us floor); the
schedule keeps DMA streaming while DVE/ACT (the only engines that can read
PSUM) carry the unavoidable per-element work:
  - all 8 x-tile loads issue up-front on the sync HWDGE ring
  - group sums of x (for the mean) come from PE mask-matmuls accumulated in
    PSUM as tiles land -- this runs inside the load window on the otherwise
    idle PE and ramps its p-state (2.4 GHz needs ~3us of continuous busy)
  - sums of x^2 are per-tile ops split between ACT (Square+accum_out) and
    DVE (scalar_tensor_tensor x*x with accum_out), tunable
  - stage A (vtx = vs^T x) packs 4 rank-12 strips per PSUM tile via
    tile_position; lhsT is zero-padded to 32 cols so strips cover all 128
    partitions (no uninitialized-PSUM reads); one wide DVE copy evacuates
    each column-half; a small DMA restores the "ones" rows that carry cst
  - stage B units of [128,1024] are routed per-unit to one of three paths:
      D: DVE scalar_tensor_tensor reads PSUM directly  (s*x + (u@vtx+cst))
      A: extra PE diag(s) matmul + ACT Identity+bias(cst)
      E: DVE 4x-mode t=s*x, ACT evac of pm (cst via ones-row), Pool TT-add
  - each unit's [128,1024] output store DMAs immediately on the sync ring
"""

from contextlib import ExitStack

import numpy as np

import concourse.bass as bass
import concourse.tile as tile
from concourse import bacc, mybir
from concourse.bass_utils import run_bass_kernel_spmd

N_CORES = 8
B, C, H, W = 16, 256, 64, 64
HW = H * W            # 4096
R = 12                # low rank
G = 32                # groups
GPC = C // G          # 8 channels per group
P = 128               # partitions
CB = C // P           # 2 channel blocks
BPC = B // N_CORES    # 2 batch elements per core
EPS = 1e-6
F32 = mybir.dt.float32
F16 = mybir.dt.float16

_MULT = mybir.AluOpType.mult
_ADD = mybir.AluOpType.add
AF = mybir.ActivationFunctionType

# ---- schedule knobs ----
# s2 (sum of squares) engine per (b, cb, tile): 'act' = ACT Square+accum,
# 'stt' = DVE STT((x*1)*x)+accum
DEF_S2 = {
    (0, 0): ("act", "stt"),
    (0, 1): ("act", "stt"),
    (1, 0): ("act", "stt"),
    (1, 1): ("act", "stt"),
}
# stage-B path per batch: 8 chars, unit order u = 2k+cb (k-major)
DEF_PATHS = ("ADAEADAE", "ADEADEAE")


def build_nc(paths=DEF_PATHS, s2cfg=DEF_S2):
    nc = bacc.Bacc(None, target_bir_lowering=False)
    x_d = nc.dram_tensor("x", [BPC, C, HW], F16, kind="ExternalInput")
    aug_d = nc.dram_tensor("aug", [BPC, P, C], F16, kind="ExternalInput")
    vsh_d = nc.dram_tensor("vsh", [BPC, CB, P, 33], F32, kind="ExternalInput")
    gmask_d = nc.dram_tensor("gmask", [P, 16], F32, kind="ExternalInput")
    gmaskT_d = nc.dram_tensor("gmaskT", [16, P], F32, kind="ExternalInput")
    ident_d = nc.dram_tensor("ident16", [P, P], F16, kind="ExternalInput")
    ones_d = nc.dram_tensor("ones16", [4, 512], F16, kind="ExternalInput")
    out_d = nc.dram_tensor("out", [BPC, C, HW], F16, kind="ExternalOutput")

    with tile.TileContext(nc) as tc, ExitStack() as ctx:
        consts = ctx.enter_context(tc.tile_pool(name="consts", bufs=1))
        xbp = ctx.enter_context(tc.tile_pool(name="xbp", bufs=8))
        junkp = ctx.enter_context(tc.tile_pool(name="junkp", bufs=2))
        outp = ctx.enter_context(tc.tile_pool(name="outp", bufs=6))
        vtp = ctx.enter_context(tc.tile_pool(name="vtp", bufs=4))
        smalls = ctx.enter_context(tc.tile_pool(name="smalls", bufs=2))
        ps_small = ctx.enter_context(
            tc.tile_pool(name="ps_small", bufs=1, space="PSUM"))
        ps_gx = ctx.enter_context(tc.tile_pool(name="ps_gx", bufs=1, space="PSUM"))
        ps_vtx = ctx.enter_context(tc.tile_pool(name="ps_vtx", bufs=1, space="PSUM"))
        ps_pm = ctx.enter_context(tc.tile_pool(name="ps_pm", bufs=2, space="PSUM"))

        # ---- consts ----
        gmask16 = consts.tile([P, 16], F16)
        nc.gpsimd.dma_start(out=gmask16, in_=gmask_d[:, :])
        gmaskF = consts.tile([P, 16], F32)
        nc.gpsimd.dma_start(out=gmaskF, in_=gmask_d[:, :])
        gmaskT = consts.tile([16, P], F32)
        nc.gpsimd.dma_start(out=gmaskT, in_=gmaskT_d[:, :])
        ident_h = consts.tile([P, P], F16)
        nc.gpsimd.dma_start(out=ident_h, in_=ident_d[:, :])
        ones4 = consts.tile([4, 512], F16)
        nc.gpsimd.dma_start(out=ones4, in_=ones_d[:, :])
        ones14 = consts.tile([1, 4], F16)
        nc.gpsimd.dma_start(out=ones14, in_=ones_d[0:1, 0:4])
        eps_t = consts.tile([16, 1], F32)
        nc.vector.memset(eps_t, EPS)

        # per-batch params on the scalar HWDGE ring (issued before x loads)
        augs, vshs = [], []
        for b in range(BPC):
            aug = smalls.tile([P, 2 * P], F16, tag=f"aug{b}", bufs=1)
            nc.scalar.dma_start(out=aug, in_=aug_d[b])
            vsh = smalls.tile([P, 66], F32, tag=f"vsh{b}", bufs=1)
            for cb in range(CB):
                nc.scalar.dma_start(
                    out=vsh[:, 33 * cb:33 * (cb + 1)], in_=vsh_d[b, cb])
            augs.append(aug)
            vshs.append(vsh)

        # warm ACT tables early so Sqrt/Square don't table-load mid-kernel
        twarm = smalls.tile([16, 1], F32, tag="twarm", bufs=1)
        nc.scalar.activation(out=twarm, in_=eps_t, func=AF.Square)
        nc.scalar.activation(out=twarm, in_=eps_t, func=AF.Sqrt,
                             bias=eps_t[:, 0:1], scale=1.0)
        nc.scalar.activation(out=twarm, in_=eps_t, func=AF.Identity)

        # ---- all x loads up-front, sync ring ----
        xbt = {}
        for b in range(BPC):
            for cb in range(CB):
                for h in range(2):
                    tb = xbp.tile([P, 2048], F16, tag="xbt")
                    nc.sync.dma_start(
                        out=tb,
                        in_=x_d[b, cb * P:(cb + 1) * P, h * 2048:(h + 1) * 2048])
                    xbt[(b, cb, h)] = tb

        sms = {}     # (b,cb) -> [128,2] f32: col0 rstd, col1 group-mean
        vss = {}     # (b,cb) -> [128,32] f16 (v*s, zero-padded)
        diags = {}   # (b,cb) -> [128,128] f16 diag(s)
        kvsb = {}    # (b,cb) -> [R,1] f32 partial kvec
        accs = {}    # (b,cb) -> [128,2] f32 sumsq per tile
        gxs = {}     # (b,cb) -> [16,512] PSUM group partial sums
        csts = {}
        vts = {}
        vtx_ps = {}

        def emit_s1mms(b, cb, t):
            # group partial sums on the PE: Gx[g,j] += sum_{c in g} x[c,512k+j]
            if t == 0:
                gxs[(b, cb)] = ps_gx.tile([16, 512], F32, tag="gx",
                                          name=f"gx{b}{cb}")
            gx = gxs[(b, cb)]
            for q in range(4):
                nc.tensor.matmul(
                    gx, lhsT=gmask16, rhs=xbt[(b, cb, t)][:, 512 * q:512 * (q + 1)],
                    start=(t == 0 and q == 0), stop=(t == 1 and q == 3),
                    skip_group_check=True)

        def emit_s2(b, cb, t):
            if (b, cb) not in accs:
                accs[(b, cb)] = smalls.tile(
                    [P, 2], F32, tag=f"acc{b}{cb}", bufs=1, name=f"acc{b}{cb}")
            acc = accs[(b, cb)]
            xt = xbt[(b, cb, t)]
            if s2cfg[(b, cb)][t] == "act":
                ja = junkp.tile([P, 2048], F16, tag="ja")
                nc.scalar.activation(out=ja, in_=xt, func=AF.Square,
                                     accum_out=acc[:, t:t + 1])
            else:
                jd = junkp.tile([P, 2048], F16, tag="jd")
                nc.vector.scalar_tensor_tensor(
                    out=jd, in0=xt, scalar=1.0, in1=xt,
                    op0=_MULT, op1=_MULT, accum_out=acc[:, t:t + 1])

        def emit_chain(b, cb):
            # group stats -> per-channel rstd/mean -> vs/diag/kvec partial
            gxred = smalls.tile([16, 1], F32, tag="gxred")
            nc.vector.tensor_reduce(
                out=gxred, in_=gxs[(b, cb)], axis=mybir.AxisListType.X,
                op=_ADD)
            acc = accs[(b, cb)]
            msum = smalls.tile([P, 1], F32, tag="msum")
            nc.vector.tensor_scalar(
                out=msum, in0=acc[:, 0:1], scalar1=acc[:, 1:2],
                scalar2=1.0 / HW, op0=_ADD, op1=_MULT)
            gs = ps_small.tile([16, 1], F32, tag="ps")
            nc.tensor.matmul(gs, lhsT=gmaskF, rhs=msum, start=True, stop=True)
            gvals = smalls.tile([16, 2], F32, tag="gvals")
            tmpg = smalls.tile([16, 4], F32, tag="tmpg")
            nc.vector.tensor_scalar_mul(
                out=gvals[:, 1:2], in0=gxred, scalar1=1.0 / (GPC * HW))
            nc.vector.tensor_scalar_mul(
                out=tmpg[:, 0:1], in0=gs, scalar1=1.0 / GPC)
            nc.vector.tensor_mul(
                out=tmpg[:, 1:2], in0=gvals[:, 1:2], in1=gvals[:, 1:2])
            nc.vector.tensor_sub(
                out=tmpg[:, 2:3], in0=tmpg[:, 0:1], in1=tmpg[:, 1:2])
            nc.scalar.activation(
                out=tmpg[:, 3:4], in_=tmpg[:, 2:3], func=AF.Sqrt,
                bias=eps_t[:, 0:1], scale=1.0)
            nc.vector.reciprocal(out=gvals[:, 0:1], in_=tmpg[:, 3:4])
            bc = ps_small.tile([P, 2], F32, tag="ps")
            nc.tensor.matmul(bc, lhsT=gmaskT, rhs=gvals, start=True, stop=True)
            sm = smalls.tile([P, 2], F32, tag=f"sm{b}{cb}", bufs=1)
            nc.vector.tensor_copy(out=sm, in_=bc)
            sms[(b, cb)] = sm
            # vs = v * s, zero-padded to 32 cols so stage A strips cover all
            # 128 partitions (evac never reads uninitialized PSUM)
            vsf = smalls.tile([P, 32], F32, tag=f"vsf{b}{cb}", bufs=1)
            nc.vector.tensor_scalar_mul(
                out=vsf, in0=vshs[b][:, 33 * cb:33 * cb + 32],
                scalar1=sm[:, 0:1])
            vs = smalls.tile([P, 32], F16, tag=f"vs{b}{cb}", bufs=1)
            nc.vector.tensor_copy(out=vs, in_=vsf)
            vss[(b, cb)] = vs
            if "A" in paths[b]:
                diag = smalls.tile([P, P], F16, tag=f"diag{b}{cb}", bufs=1)
                nc.vector.tensor_scalar_mul(out=diag, in0=ident_h,
                                            scalar1=sm[:, 0:1])
                diags[(b, cb)] = diag
            # kvec partial: kv[r] = sum_c vs[c,r]*m_c
            kv = ps_small.tile([32, 1], F32, tag="ps")
            nc.tensor.matmul(kv, lhsT=vsf, rhs=sm[:, 1:2], start=True, stop=True)
            kvp = smalls.tile([R, 1], F32, tag=f"kv{b}{cb}", bufs=1)
            nc.vector.tensor_copy(out=kvp, in_=kv[0:R, :])
            kvsb[(b, cb)] = kvp

        def emit_cst(b):
            # cst_c = shift_c - m_c*s_c - sum_r u[c,r]*kvec[r]; broadcast into
            # aug rows 32s+12 (all 4 strips) via transpose + SBUF->SBUF DMA
            kvs = smalls.tile([R, 1], F32, tag=f"kvs{b}", bufs=1)
            nc.vector.tensor_add(out=kvs, in0=kvsb[(b, 0)], in1=kvsb[(b, 1)])
            kvs16 = smalls.tile([R, 1], F16, tag=f"kvs16{b}", bufs=1)
            nc.vector.tensor_copy(out=kvs16, in_=kvs)
            aug = augs[b]
            for cb in range(CB):
                sm = sms[(b, cb)]
                ukv = ps_small.tile([P, 1], F32, tag="ps")
                nc.tensor.matmul(
                    ukv, lhsT=aug[0:R, P * cb:P * (cb + 1)], rhs=kvs16,
                    start=True, stop=True)
                cst = smalls.tile([P, 1], F32, tag=f"cst{b}{cb}", bufs=1)
                nc.vector.tensor_mul(out=cst, in0=sm[:, 1:2], in1=sm[:, 0:1])
                nc.vector.tensor_sub(
                    out=cst, in0=vshs[b][:, 33 * cb + 32:33 * cb + 33], in1=cst)
                nc.vector.tensor_sub(out=cst, in0=cst, in1=ukv)
                csts[(b, cb)] = cst
                cst16 = smalls.tile([P, 1], F16, tag="cst16")
                nc.vector.tensor_copy(out=cst16, in_=cst)
                ctp = ps_small.tile([1, P], F16, tag="ps")
                nc.tensor.transpose(out=ctp, in_=cst16, identity=ident_h)
                cstrow = smalls.tile([1, P], F16, tag="cstrow")
                nc.scalar.copy(out=cstrow, in_=ctp)
                ctp4 = ps_small.tile([4, P], F32, tag="ps")
                nc.tensor.matmul(ctp4, lhsT=ones14, rhs=cstrow,
                                 start=True, stop=True)
                cstrow4 = smalls.tile([4, P], F16, tag="cstrow4")
                nc.scalar.copy(out=cstrow4, in_=ctp4)
                pstride = aug.ap[0][0]
                dst = bass.AP(
                    tensor=aug.tensor,
                    offset=aug.offset + R * pstride + P * cb,
                    ap=[[32 * pstride, 4], [1, P]])
                nc.gpsimd.dma_start(out=dst, in_=cstrow4)

        def emit_stage_a(b, ch):
            # vtx strips for chunks j = 4*ch + q; strip q covers partitions
            # 32q..32q+31 (rows 12..31 are zeros from the padded lhsT)
            if ch == 0:
                vtx_ps[b] = ps_vtx.tile([P, 1024], F32, tag="vtx",
                                        name=f"vtx{b}")
            vps = vtx_ps[b]
            for cb in range(CB):
                for q in range(4):
                    nc.tensor.matmul(
                        vps[32 * q:32 * q + 32, 512 * ch:512 * (ch + 1)],
                        lhsT=vss[(b, cb)],
                        rhs=xbt[(b, cb, ch)][:, 512 * q:512 * (q + 1)],
                        start=(cb == 0), stop=(cb == CB - 1),
                        tile_position=(0, 32 * q),
                        skip_group_check=True)

        def emit_evac(b, ch):
            vt = vtp.tile([P, 512], F16, tag="vt")
            nc.vector.tensor_copy(
                out=vt, in_=vtx_ps[b][:, 512 * ch:512 * (ch + 1)])
            # restore ones rows (partitions 12,44,76,108) clobbered by the
            # full-tile copy; engines can't write at partition offset 12, DMA can
            pstride = vt.ap[0][0]
            dst = bass.AP(
                tensor=vt.tensor,
                offset=vt.offset + 12 * pstride,
                ap=[[32 * pstride, 4], [1, 512]])
            nc.gpsimd.dma_start(out=dst, in_=ones4[:, :])
            vts[(b, ch)] = vt

        def emit_unit(b, k, cb):
            # output unit [128,1024]: chunks (2k, 2k+1); x tile h=k//2,
            # cols (k%2)*1024; vtx strips q = 2k%4, (2k+1)%4 of vts[(b, k//2)]
            path = paths[b][2 * k + cb]
            h, half = k // 2, k % 2
            aug = augs[b]
            sm = sms[(b, cb)]
            x_ap = xbt[(b, cb, h)][:, 1024 * half:1024 * (half + 1)]
            pm = ps_pm.tile([P, 1024], F32, tag="pm")
            vt = vts[(b, h)]
            for j2 in range(2):
                q = 2 * half + j2
                pslice = pm[:, 512 * j2:512 * (j2 + 1)]
                if path == "A":
                    nc.tensor.matmul(
                        pslice, lhsT=diags[(b, cb)],
                        rhs=xbt[(b, cb, h)][:, 512 * q:512 * (q + 1)],
                        start=True, stop=False,
                        skip_group_check=True)
                    nc.tensor.matmul(
                        pslice,
                        lhsT=aug[32 * q:32 * q + R, P * cb:P * (cb + 1)],
                        rhs=vt[32 * q:32 * q + R, :],
                        start=False, stop=True,
                        tile_position=(32 * q, 0),
                        skip_group_check=True)
                else:
                    nc.tensor.matmul(
                        pslice,
                        lhsT=aug[32 * q:32 * q + R + 1, P * cb:P * (cb + 1)],
                        rhs=vt[32 * q:32 * q + R + 1, :],
                        start=True, stop=True,
                        tile_position=(32 * q, 0),
                        skip_group_check=True)
            osb = outp.tile([P, 1024], F16, tag="osb")
            if path == "A":
                nc.scalar.activation(
                    out=osb, in_=pm, func=AF.Identity,
                    bias=csts[(b, cb)], scale=1.0)
            elif path == "D":
                nc.vector.scalar_tensor_tensor(
                    out=osb, in0=x_ap, scalar=sm[:, 0:1], in1=pm,
                    op0=_MULT, op1=_ADD)
            else:  # E: DVE 4x t=s*x, ACT evac (cst already in pm), Pool add
                t = outp.tile([P, 1024], F16, tag="tsx", bufs=3)
                nc.vector.tensor_scalar(
                    out=t, in0=x_ap, scalar1=sm[:, 0:1], scalar2=0.0,
                    op0=_MULT, op1=_ADD)
                pmsb = outp.tile([P, 1024], F16, tag="pmsb", bufs=3)
                nc.scalar.activation(out=pmsb, in_=pm, func=AF.Identity)
                nc.gpsimd.tensor_add(out=osb, in0=t, in1=pmsb)
            nc.sync.dma_start(
                out=out_d[b, cb * P:(cb + 1) * P, 1024 * k:1024 * (k + 1)],
                in_=osb)

        # ================= schedule =================
        for cb in range(CB):
            emit_s1mms(0, cb, 0)
            emit_s2(0, cb, 0)
            emit_s1mms(0, cb, 1)
            emit_s2(0, cb, 1)
            emit_chain(0, cb)
        emit_cst(0)
        emit_s1mms(1, 0, 0)
        emit_s2(1, 0, 0)
        emit_s1mms(1, 0, 1)
        emit_s2(1, 0, 1)
        emit_stage_a(0, 0)
        emit_evac(0, 0)
        emit_s1mms(1, 1, 0)
        emit_s2(1, 1, 0)
        emit_s1mms(1, 1, 1)
        emit_s2(1, 1, 1)
        emit_stage_a(0, 1)
        emit_evac(0, 1)
        emit_chain(1, 0)
        for k in range(2):
            for cb in range(CB):
                emit_unit(0, k, cb)
        emit_chain(1, 1)
        emit_cst(1)
        for k in range(2, 4):
            for cb in range(CB):
                emit_unit(0, k, cb)
        emit_stage_a(1, 0)
        emit_evac(1, 0)
        emit_stage_a(1, 1)
        emit_evac(1, 1)
        for k in range(4):
            for cb in range(CB):
                emit_unit(1, k, cb)

    nc.finalize()
    return nc


def _host_prep(x, ccm_params):
    x = np.asarray(x, dtype=np.float32).reshape(B, C, HW).astype(np.float16)
    x = np.ascontiguousarray(x)
    cp = np.asarray(ccm_params, dtype=np.float32)
    u = cp[:, :C * R].reshape(B, C, R)
    v = cp[:, C * R:2 * C * R].reshape(B, C, R)
    shift = cp[:, 2 * C * R:].reshape(B, C)
    # aug: [B, 128, C] fp16; strips s=0..3: rows 32s..32s+11 = u^T,
    # row 32s+12 = cst written on device
    aug = np.zeros((B, P, C), np.float16)
    ut = u.transpose(0, 2, 1).astype(np.float16)
    for sx in range(4):
        aug[:, 32 * sx:32 * sx + R, :] = ut
    aug = np.ascontiguousarray(aug)
    # vsh: [B, CB, P, 33] f32: cols 0..11 = v, 12..31 zero pad, col 32 = shift
    vsh = np.zeros((B, CB, P, 33), np.float32)
    vsh[..., :R] = v.reshape(B, CB, P, R)
    vsh[..., 32] = shift.reshape(B, CB, P)
    vsh = np.ascontiguousarray(vsh)
    gmask = np.zeros((P, 16), np.float32)
    gmask[np.arange(P), np.arange(P) // GPC] = 1.0
    gmaskT = np.ascontiguousarray(gmask.T)
    ident16 = np.eye(P, dtype=np.float16)
    ones16 = np.ones((4, 512), np.float16)
    in_maps = []
    for c in range(N_CORES):
        bs = slice(c * BPC, (c + 1) * BPC)
        in_maps.append({
            "x": x[bs], "aug": aug[bs], "vsh": vsh[bs],
            "gmask": gmask, "gmaskT": gmaskT, "ident16": ident16,
            "ones16": ones16,
        })
    return in_maps


def kernel(x, ccm_params, _trace=False, _paths=DEF_PATHS, _s2=DEF_S2,
           **_ignored):
    in_maps = _host_prep(x, ccm_params)
    nc = build_nc(paths=_paths, s2cfg=_s2)
    res = run_bass_kernel_spmd(
        nc, in_maps, core_ids=list(range(N_CORES)), trace=_trace)
    out = np.concatenate([r["out"] for r in res.results], axis=0)
    out = out.reshape(B, C, H, W).astype(np.float32, copy=False)
    if _trace:
        return out, res
    return out


# revision 13
# speedup vs baseline: 1.2452x; 1.1703x over previous
"""CCMLite kernel for Trainium2: GroupNorm(affine=False) + low-rank channel mix.

out = x_norm + u @ (v^T @ x_norm) + shift, with x_norm = groupnorm(x).

Sharding: data-parallel over batch B=16 across 8 cores (2 batch elems/core).

DMA floor is ~24us/core (8.4 MB at ~360 GB/s); DVE/ACT are the only engines
that can read PSUM, so the schedule spreads the unavoidable per-element work:
  - all x loads issue up-front on the sync ring (last tile split in half so
    tail stats start earlier)
  - batch-0 group sums (mean) ride PE mask-matmuls inside the load window;
    batch-1 sums go to DVE tensor_reduce / ACT Copy+accum per config
  - sums of squares per tile on ACT (Square+accum) or DVE (STT x*x+accum)
  - stage A (vtx = vs^T x) packs 4 rank-12 strips per PSUM tile via
    tile_position; lhsT zero-padded to 32 cols so strips cover all 128
    partitions; the evacuation op subtracts kvec (the mean correction of
    vtx) as a per-partition scalar, and maps the zero rows 32q+12 to +1
    (via kvecP rows = -1), providing the ones-row that carries cst through
    stage-B matmuls -- no separate correction matmuls or ones restores
  - stage B units of [128,1024]: path D = DVE STT (s*x + pm) from PSUM,
    path A = PE diag(s) matmul + ACT Identity+bias, path E = DVE 4x t=s*x,
    ACT evac, Pool TT-add; per-unit stores stream on the sync ring
"""

from contextlib import ExitStack

import numpy as np

import concourse.bass as bass
import concourse.tile as tile
from concourse import bacc, mybir
from concourse.bass_utils import run_bass_kernel_spmd

N_CORES = 8
B, C, H, W = 16, 256, 64, 64
HW = H * W            # 4096
R = 12                # low rank
G = 32                # groups
GPC = C // G          # 8 channels per group
P = 128               # partitions
CB = C // P           # 2 channel blocks
BPC = B // N_CORES    # 2 batch elements per core
EPS = 1e-6
F32 = mybir.dt.float32
F16 = mybir.dt.float16

_MULT = mybir.AluOpType.mult
_ADD = mybir.AluOpType.add
AF = mybir.ActivationFunctionType

# ---- schedule knobs ----
# s1 (sum of x) method per (b, cb): 'pe' = PE mask-matmuls into PSUM (group
# level; only legal for b=0 -- shares the PSUM ring with vtx), 'red' = DVE
# tensor_reduce per tile, 'cpy' = ACT Copy+accum per tile.
DEF_S1 = {(0, 0): "pe", (0, 1): "pe", (1, 0): "red", (1, 1): "cpy"}
# s2 engine per (b, cb, tile): 'act' = ACT Square+accum, 'stt' = DVE STT
DEF_S2 = {
    (0, 0): ("act", "act"),
    (0, 1): ("act", "act"),
    (1, 0): ("act", "act"),
    (1, 1): ("act", "stt"),
}
# stage-B path per batch: 8 chars, unit order u = 2k+cb (k-major)
DEF_PATHS = ("DEDEDEDE", "DDEDDEDE")


def build_nc(paths=DEF_PATHS, s1cfg=DEF_S1, s2cfg=DEF_S2):
    nc = bacc.Bacc(None, target_bir_lowering=False)
    x_d = nc.dram_tensor("x", [BPC, C, HW], F16, kind="ExternalInput")
    aug_d = nc.dram_tensor("aug", [BPC, P, C], F16, kind="ExternalInput")
    vsh_d = nc.dram_tensor("vsh", [BPC, CB, P, 33], F32, kind="ExternalInput")
    gmask_d = nc.dram_tensor("gmask", [P, 16], F32, kind="ExternalInput")
    gmaskT_d = nc.dram_tensor("gmaskT", [16, P], F32, kind="ExternalInput")
    ident_d = nc.dram_tensor("ident16", [P, P], F16, kind="ExternalInput")
    kinit_d = nc.dram_tensor("kinit", [16, 1], F32, kind="ExternalInput")
    ones_d = nc.dram_tensor("ones16", [1, 4], F16, kind="ExternalInput")
    out_d = nc.dram_tensor("out", [BPC, C, HW], F16, kind="ExternalOutput")

    with tile.TileContext(nc) as tc, ExitStack() as ctx:
        consts = ctx.enter_context(tc.tile_pool(name="consts", bufs=1))
        xbp = ctx.enter_context(tc.tile_pool(name="xbp", bufs=9))
        junkp = ctx.enter_context(tc.tile_pool(name="junkp", bufs=2))
        outp = ctx.enter_context(tc.tile_pool(name="outp", bufs=6))
        vtp = ctx.enter_context(tc.tile_pool(name="vtp", bufs=4))
        smalls = ctx.enter_context(tc.tile_pool(name="smalls", bufs=2))
        ps_small = ctx.enter_context(
            tc.tile_pool(name="ps_small", bufs=2, space="PSUM"))
        # big ring: gx(0,0), gx(0,1), vtx0, vtx1 have sequential lifetimes
        ps_big = ctx.enter_context(tc.tile_pool(name="ps_big", bufs=1, space="PSUM"))
        ps_pm = ctx.enter_context(tc.tile_pool(name="ps_pm", bufs=2, space="PSUM"))

        # ---- consts ----
        gmask16 = consts.tile([P, 16], F16)
        nc.gpsimd.dma_start(out=gmask16, in_=gmask_d[:, :])
        gmaskF = consts.tile([P, 16], F32)
        nc.gpsimd.dma_start(out=gmaskF, in_=gmask_d[:, :])
        gmaskT = consts.tile([16, P], F32)
        nc.gpsimd.dma_start(out=gmaskT, in_=gmaskT_d[:, :])
        ident_h = consts.tile([P, P], F16)
        nc.gpsimd.dma_start(out=ident_h, in_=ident_d[:, :])
        kinit = consts.tile([16, 1], F32)
        nc.gpsimd.dma_start(out=kinit, in_=kinit_d[:, :])
        ones14 = consts.tile([1, 4], F16)
        nc.gpsimd.dma_start(out=ones14, in_=ones_d[:, :])
        eps_t = consts.tile([16, 1], F32)
        nc.vector.memset(eps_t, EPS)

        # per-batch params on the scalar HWDGE ring (issued before x loads)
        augs, vshs = [], []
        for b in range(BPC):
            aug = smalls.tile([P, 2 * P], F16, tag=f"aug{b}", bufs=1)
            nc.scalar.dma_start(out=aug, in_=aug_d[b])
            vsh = smalls.tile([P, 66], F32, tag=f"vsh{b}", bufs=1)
            for cb in range(CB):
                nc.scalar.dma_start(
                    out=vsh[:, 33 * cb:33 * (cb + 1)], in_=vsh_d[b, cb])
            augs.append(aug)
            vshs.append(vsh)

        # warm ACT tables early so Sqrt/Square don't table-load mid-kernel
        twarm = smalls.tile([16, 1], F32, tag="twarm", bufs=1)
        nc.scalar.activation(out=twarm, in_=eps_t, func=AF.Square)
        nc.scalar.activation(out=twarm, in_=eps_t, func=AF.Sqrt,
                             bias=eps_t[:, 0:1], scale=1.0)
        nc.scalar.activation(out=twarm, in_=eps_t, func=AF.Identity)

        # ---- all x loads up-front, sync ring; the LAST tile is split so the
        # tail-gating stats can start ~1.5us earlier ----
        # xparts[(b,cb,h)] = list of (tile, col0, width) covering cols 0..2048
        xparts = {}
        for b in range(BPC):
            for cb in range(CB):
                for h in range(2):
                    if (b, cb, h) == (1, 1, 1):
                        parts = []
                        for i in range(2):
                            tb = xbp.tile([P, 1024], F16, tag="xbt2",
                                          bufs=2, name=f"xl{i}")
                            nc.sync.dma_start(
                                out=tb,
                                in_=x_d[b, cb * P:(cb + 1) * P,
                                        h * 2048 + 1024 * i:
                                        h * 2048 + 1024 * (i + 1)])
                            parts.append((tb, 1024 * i, 1024))
                        xparts[(b, cb, h)] = parts
                    else:
                        tb = xbp.tile([P, 2048], F16, tag="xbt",
                                      name=f"x{b}{cb}{h}")
                        nc.sync.dma_start(
                            out=tb,
                            in_=x_d[b, cb * P:(cb + 1) * P,
                                    h * 2048:(h + 1) * 2048])
                        xparts[(b, cb, h)] = [(tb, 0, 2048)]

        def x_ap(b, cb, col0, width):
            # AP for x cols [col0, col0+width) of batch b, channel block cb
            h, c = col0 // 2048, col0 % 2048
            for tile_, t0, tw in xparts[(b, cb, h)]:
                if t0 <= c and c + width <= t0 + tw:
                    return tile_[:, c - t0:c - t0 + width]
            raise AssertionError((b, cb, col0, width))

        sms = {}     # (b,cb) -> [128,2] f32: col0 rstd, col1 group-mean
        vss = {}     # (b,cb) -> [128,32] f16 (v*s, zero-padded)
        diags = {}   # (b,cb) -> [128,128] f16 diag(s)
        kvsb = {}    # (b,cb) -> [R,1] f32 partial kvec
        kvecPs = {}  # b -> [128,1] f32 strip-replicated kvec (rows 32q+12=-1)
        accs = {}    # (b,cb) -> [128,8] f32 accumulators
        s1cols = {}  # (b,cb) -> number of per-channel s1 cols used
        s2cols = {}  # (b,cb) -> number of s2 cols used
        gxs = {}     # (b,cb) -> [16,512] PSUM group partial sums
        csts = {}
        vts = {}
        vtx_ps = {}

        def get_acc(b, cb):
            if (b, cb) not in accs:
                accs[(b, cb)] = smalls.tile(
                    [P, 8], F32, tag=f"acc{b}{cb}", bufs=1, name=f"acc{b}{cb}")
                s1cols[(b, cb)] = 0
                s2cols[(b, cb)] = 0
            return accs[(b, cb)]

        def emit_s1(b, cb, h):
            # sum of x over this [*,2048] column range
            mode = s1cfg[(b, cb)]
            if mode == "pe":
                if h == 0:
                    gxs[(b, cb)] = ps_big.tile([16, 512], F32, tag="big",
                                               name=f"gx{b}{cb}")
                gx = gxs[(b, cb)]
                for q in range(4):
                    nc.tensor.matmul(
                        gx, lhsT=gmask16,
                        rhs=x_ap(b, cb, 2048 * h + 512 * q, 512),
                        start=(h == 0 and q == 0), stop=(h == 1 and q == 3),
                        skip_group_check=True)
                return
            acc = get_acc(b, cb)
            for tile_, t0, tw in xparts[(b, cb, h)]:
                col = s1cols[(b, cb)]
                s1cols[(b, cb)] += 1
                if mode == "red":
                    nc.vector.tensor_reduce(
                        out=acc[:, col:col + 1], in_=tile_,
                        axis=mybir.AxisListType.X, op=_ADD)
                else:  # cpy
                    jc = junkp.tile([P, 2048], F16, tag="jc")
                    nc.scalar.activation(
                        out=jc[:, 0:tw], in_=tile_, func=AF.Copy,
                        accum_out=acc[:, col:col + 1])

        def emit_s2(b, cb, h):
            acc = get_acc(b, cb)
            for tile_, t0, tw in xparts[(b, cb, h)]:
                col = 4 + s2cols[(b, cb)]
                s2cols[(b, cb)] += 1
                if s2cfg[(b, cb)][h] == "act":
                    ja = junkp.tile([P, 2048], F16, tag="ja")
                    nc.scalar.activation(out=ja[:, 0:tw], in_=tile_,
                                         func=AF.Square,
                                         accum_out=acc[:, col:col + 1])
                else:
                    jd = junkp.tile([P, 2048], F16, tag="jd")
                    nc.vector.scalar_tensor_tensor(
                        out=jd[:, 0:tw], in0=tile_, scalar=1.0, in1=tile_,
                        op0=_MULT, op1=_MULT, accum_out=acc[:, col:col + 1])

        def fold(acc, c0, n, out):
            # out = sum(acc[:, c0:c0+n]) / HW
            if n == 2:
                nc.vector.tensor_scalar(
                    out=out, in0=acc[:, c0:c0 + 1], scalar1=acc[:, c0 + 1:c0 + 2],
                    scalar2=1.0 / HW, op0=_ADD, op1=_MULT)
            else:
                assert n == 3
                nc.vector.tensor_scalar(
                    out=acc[:, c0:c0 + 1], in0=acc[:, c0:c0 + 1],
                    scalar1=acc[:, c0 + 1:c0 + 2], scalar2=acc[:, c0 + 2:c0 + 3],
                    op0=_ADD, op1=_ADD)
                nc.vector.tensor_scalar_mul(
                    out=out, in0=acc[:, c0:c0 + 1], scalar1=1.0 / HW)

        def emit_chain(b, cb):
            # group stats -> per-channel rstd/mean -> vs/diag/kvec partial
            acc = accs.get((b, cb))
            gvals = smalls.tile([16, 2], F32, tag="gvals")
            tmpg = smalls.tile([16, 4], F32, tag="tmpg")
            if s1cfg[(b, cb)] == "pe":
                gxred = smalls.tile([16, 1], F32, tag="gxred")
                nc.vector.tensor_reduce(
                    out=gxred, in_=gxs[(b, cb)], axis=mybir.AxisListType.X,
                    op=_ADD)
                # per-channel E[x^2] -> group
                msum = smalls.tile([P, 1], F32, tag="msum")
                fold(acc, 4, s2cols[(b, cb)], msum)
                gs = ps_small.tile([16, 1], F32, tag="ps")
                nc.tensor.matmul(gs, lhsT=gmaskF, rhs=msum,
                                 start=True, stop=True)
                nc.vector.tensor_scalar_mul(
                    out=gvals[:, 1:2], in0=gxred, scalar1=1.0 / (GPC * HW))
                nc.vector.tensor_scalar_mul(
                    out=tmpg[:, 0:1], in0=gs, scalar1=1.0 / GPC)
            else:
                # per-channel s1 in acc cols 0..n-1, s2 in cols 2..3
                msum = smalls.tile([P, 2], F32, tag="msum2")
                fold(acc, 0, s1cols[(b, cb)], msum[:, 0:1])
                fold(acc, 4, s2cols[(b, cb)], msum[:, 1:2])
                gs = ps_small.tile([16, 2], F32, tag="ps")
                nc.tensor.matmul(gs, lhsT=gmaskF, rhs=msum,
                                 start=True, stop=True)
                nc.vector.tensor_scalar_mul(
                    out=gvals[:, 1:2], in0=gs[:, 0:1], scalar1=1.0 / GPC)
                nc.vector.tensor_scalar_mul(
                    out=tmpg[:, 0:1], in0=gs[:, 1:2], scalar1=1.0 / GPC)
            nc.vector.tensor_mul(
                out=tmpg[:, 1:2], in0=gvals[:, 1:2], in1=gvals[:, 1:2])
            nc.vector.tensor_sub(
                out=tmpg[:, 2:3], in0=tmpg[:, 0:1], in1=tmpg[:, 1:2])
            nc.scalar.activation(
                out=tmpg[:, 3:4], in_=tmpg[:, 2:3], func=AF.Sqrt,
                bias=eps_t[:, 0:1], scale=1.0)
            nc.vector.reciprocal(out=gvals[:, 0:1], in_=tmpg[:, 3:4])
            bc = ps_small.tile([P, 2], F32, tag="ps")
            nc.tensor.matmul(bc, lhsT=gmaskT, rhs=gvals, start=True, stop=True)
            sm = smalls.tile([P, 2], F32, tag=f"sm{b}{cb}", bufs=1)
            nc.vector.tensor_copy(out=sm, in_=bc)
            sms[(b, cb)] = sm
            # vs = v * s, zero-padded to 32 cols so stage A strips cover all
            # 128 partitions (evac never reads uninitialized PSUM)
            vsf = smalls.tile([P, 32], F32, tag=f"vsf{b}{cb}", bufs=1)
            nc.vector.tensor_scalar_mul(
                out=vsf, in0=vshs[b][:, 33 * cb:33 * cb + 32],
                scalar1=sm[:, 0:1])
            vs = smalls.tile([P, 32], F16, tag=f"vs{b}{cb}", bufs=1)
            nc.vector.tensor_copy(out=vs, in_=vsf)
            vss[(b, cb)] = vs
            if "A" in paths[b]:
                diag = smalls.tile([P, P], F16, tag=f"diag{b}{cb}", bufs=1)
                nc.vector.tensor_scalar_mul(out=diag, in0=ident_h,
                                            scalar1=sm[:, 0:1])
                diags[(b, cb)] = diag
            # kvec partial: kv[r] = sum_c vs[c,r]*m_c
            kv = ps_small.tile([32, 1], F32, tag="ps")
            nc.tensor.matmul(kv, lhsT=vsf, rhs=sm[:, 1:2], start=True, stop=True)
            kvp = smalls.tile([R, 1], F32, tag=f"kv{b}{cb}", bufs=1)
            nc.vector.tensor_copy(out=kvp, in_=kv[0:R, :])
            kvsb[(b, cb)] = kvp

        def emit_cst(b):
            # kvecP: rows 32q+r = kvec[r], rows 32q+12 = -1 (so the evac's
            # 0 - kvecP produces the +1 ones-row), rest 0.
            krow = smalls.tile([16, 1], F32, tag=f"krow{b}", bufs=1)
            nc.gpsimd.dma_start(out=krow, in_=kinit[:, :])
            nc.vector.tensor_add(
                out=krow[0:R, :], in0=kvsb[(b, 0)], in1=kvsb[(b, 1)])
            kvecP = smalls.tile([P, 1], F32, tag=f"kvecP{b}", bufs=1)
            for q in range(4):
                nc.gpsimd.dma_start(out=kvecP[32 * q:32 * q + 16, :], in_=krow)
            kvecPs[b] = kvecP
            # cst_c = shift_c - m_c*s_c, broadcast into aug rows 32q+12
            aug = augs[b]
            for cb in range(CB):
                sm = sms[(b, cb)]
                cst = smalls.tile([P, 1], F32, tag=f"cst{b}{cb}", bufs=1)
                nc.vector.tensor_mul(out=cst, in0=sm[:, 1:2], in1=sm[:, 0:1])
                nc.vector.tensor_sub(
                    out=cst, in0=vshs[b][:, 33 * cb + 32:33 * cb + 33], in1=cst)
                csts[(b, cb)] = cst
                cst16 = smalls.tile([P, 1], F16, tag="cst16")
                nc.vector.tensor_copy(out=cst16, in_=cst)
                ctp = ps_small.tile([1, P], F16, tag="ps")
                nc.tensor.transpose(out=ctp, in_=cst16, identity=ident_h)
                cstrow = smalls.tile([1, P], F16, tag="cstrow")
                nc.scalar.copy(out=cstrow, in_=ctp)
                ctp4 = ps_small.tile([4, P], F32, tag="ps")
                nc.tensor.matmul(ctp4, lhsT=ones14, rhs=cstrow,
                                 start=True, stop=True)
                cstrow4 = smalls.tile([4, P], F16, tag="cstrow4")
                nc.scalar.copy(out=cstrow4, in_=ctp4)
                pstride = aug.ap[0][0]
                dst = bass.AP(
                    tensor=aug.tensor,
                    offset=aug.offset + R * pstride + P * cb,
                    ap=[[32 * pstride, 4], [1, P]])
                nc.gpsimd.dma_start(out=dst, in_=cstrow4)

        def emit_stage_a(b, ch):
            # vtx strips for chunks j = 4*ch + q; strip q covers partitions
            # 32q..32q+31 (rows 12..31 are zeros from the padded lhsT)
            if ch == 0:
                vtx_ps[b] = ps_big.tile([P, 1024], F32, tag="big",
                                        name=f"vtx{b}")
            vps = vtx_ps[b]
            for cb in range(CB):
                for q in range(4):
                    nc.tensor.matmul(
                        vps[32 * q:32 * q + 32, 512 * ch:512 * (ch + 1)],
                        lhsT=vss[(b, cb)],
                        rhs=x_ap(b, cb, 2048 * ch + 512 * q, 512),
                        start=(cb == 0), stop=(cb == CB - 1),
                        tile_position=(0, 32 * q),
                        skip_group_check=True)

        def emit_evac(b, ch):
            # vt = vtx - kvec (mean correction folded in); zero rows 32q+12
            # become +1 (kvecP=-1 there): the ones-row for the cst matmul row
            vt = vtp.tile([P, 512], F16, tag="vt")
            nc.vector.tensor_scalar_sub(
                out=vt, in0=vtx_ps[b][:, 512 * ch:512 * (ch + 1)],
                scalar1=kvecPs[b])
            vts[(b, ch)] = vt

        def emit_unit(b, k, cb):
            # output unit [128,1024]: chunks (2k, 2k+1); vtx strips
            # q = 2k%4, (2k+1)%4 of vts[(b, k//2)]
            path = paths[b][2 * k + cb]
            aug = augs[b]
            sm = sms[(b, cb)]
            xap = x_ap(b, cb, 1024 * k, 1024)
            pm = ps_pm.tile([P, 1024], F32, tag="pm")
            vt = vts[(b, k // 2)]
            for j2 in range(2):
                q = (2 * k + j2) % 4
                pslice = pm[:, 512 * j2:512 * (j2 + 1)]
                if path == "A":
                    nc.tensor.matmul(
                        pslice, lhsT=diags[(b, cb)],
                        rhs=x_ap(b, cb, 1024 * k + 512 * j2, 512),
                        start=True, stop=False,
                        skip_group_check=True)
                    nc.tensor.matmul(
                        pslice,
                        lhsT=aug[32 * q:32 * q + R, P * cb:P * (cb + 1)],
                        rhs=vt[32 * q:32 * q + R, :],
                        start=False, stop=True,
                        tile_position=(32 * q, 0),
                        skip_group_check=True)
                else:
                    nc.tensor.matmul(
                        pslice,
                        lhsT=aug[32 * q:32 * q + R + 1, P * cb:P * (cb + 1)],
                        rhs=vt[32 * q:32 * q + R + 1, :],
                        start=True, stop=True,
                        tile_position=(32 * q, 0),
                        skip_group_check=True)
            osb = outp.tile([P, 1024], F16, tag="osb")
            if path == "A":
                nc.scalar.activation(
                    out=osb, in_=pm, func=AF.Identity,
                    bias=csts[(b, cb)], scale=1.0)
            elif path == "D":
                nc.vector.scalar_tensor_tensor(
                    out=osb, in0=xap, scalar=sm[:, 0:1], in1=pm,
                    op0=_MULT, op1=_ADD)
            else:  # E
                t = outp.tile([P, 1024], F16, tag="tsx", bufs=3)
                nc.vector.tensor_scalar(
                    out=t, in0=xap, scalar1=sm[:, 0:1], scalar2=0.0,
                    op0=_MULT, op1=_ADD)
                pmsb = outp.tile([P, 1024], F16, tag="pmsb", bufs=3)
                nc.scalar.activation(out=pmsb, in_=pm, func=AF.Identity)
                nc.gpsimd.tensor_add(out=osb, in0=t, in1=pmsb)
            nc.sync.dma_start(
                out=out_d[b, cb * P:(cb + 1) * P, 1024 * k:1024 * (k + 1)],
                in_=osb)

        # ================= schedule =================
        for cb in range(CB):
            emit_s1(0, cb, 0)
            emit_s2(0, cb, 0)
            emit_s1(0, cb, 1)
            emit_s2(0, cb, 1)
            emit_chain(0, cb)
        emit_cst(0)
        emit_s1(1, 0, 0)
        emit_s2(1, 0, 0)
        emit_s1(1, 0, 1)
        emit_s2(1, 0, 1)
        emit_stage_a(0, 0)
        emit_evac(0, 0)
        emit_s1(1, 1, 0)
        emit_s2(1, 1, 0)
        emit_s1(1, 1, 1)
        emit_s2(1, 1, 1)
        emit_stage_a(0, 1)
        emit_evac(0, 1)
        emit_chain(1, 0)
        for k in range(2):
            for cb in range(CB):
                emit_unit(0, k, cb)
        emit_chain(1, 1)
        emit_cst(1)
        for k in range(2, 4):
            for cb in range(CB):
                emit_unit(0, k, cb)
        emit_stage_a(1, 0)
        emit_evac(1, 0)
        emit_stage_a(1, 1)
        emit_evac(1, 1)
        for k in range(4):
            for cb in range(CB):
                emit_unit(1, k, cb)

    nc.finalize()
    return nc


def _host_prep(x, ccm_params):
    x = np.asarray(x, dtype=np.float32).reshape(B, C, HW).astype(np.float16)
    x = np.ascontiguousarray(x)
    cp = np.asarray(ccm_params, dtype=np.float32)
    u = cp[:, :C * R].reshape(B, C, R)
    v = cp[:, C * R:2 * C * R].reshape(B, C, R)
    shift = cp[:, 2 * C * R:].reshape(B, C)
    # aug: [B, 128, C] fp16; strips s=0..3: rows 32s..32s+11 = u^T,
    # row 32s+12 = cst written on device
    aug = np.zeros((B, P, C), np.float16)
    ut = u.transpose(0, 2, 1).astype(np.float16)
    for sx in range(4):
        aug[:, 32 * sx:32 * sx + R, :] = ut
    aug = np.ascontiguousarray(aug)
    # vsh: [B, CB, P, 33] f32: cols 0..11 = v, 12..31 zero pad, col 32 = shift
    vsh = np.zeros((B, CB, P, 33), np.float32)
    vsh[..., :R] = v.reshape(B, CB, P, R)
    vsh[..., 32] = shift.reshape(B, CB, P)
    vsh = np.ascontiguousarray(vsh)
    gmask = np.zeros((P, 16), np.float32)
    gmask[np.arange(P), np.arange(P) // GPC] = 1.0
    gmaskT = np.ascontiguousarray(gmask.T)
    ident16 = np.eye(P, dtype=np.float16)
    kinit = np.zeros((16, 1), np.float32)
    kinit[12, 0] = -1.0
    in_maps = []
    for c in range(N_CORES):
        bs = slice(c * BPC, (c + 1) * BPC)
        in_maps.append({
            "x": x[bs], "aug": aug[bs], "vsh": vsh[bs],
            "gmask": gmask, "gmaskT": gmaskT, "ident16": ident16,
            "kinit": kinit, "ones16": np.ones((1, 4), np.float16),
        })
    return in_maps


def kernel(x, ccm_params, _trace=False, _paths=DEF_PATHS, _s1=DEF_S1,
           _s2=DEF_S2, **_ignored):
    in_maps = _host_prep(x, ccm_params)
    nc = build_nc(paths=_paths, s1cfg=_s1, s2cfg=_s2)
    res = run_bass_kernel_spmd(
        nc, in_maps, core_ids=list(range(N_CORES)), trace=_trace)
    out = np.concatenate([r["out"] for r in res.results], axis=0)
    out = out.reshape(B, C, H, W).astype(np.float32, copy=False)
    if _trace:
        return out, res
    return out


# revision 16
# speedup vs baseline: 1.2518x; 1.0053x over previous
"""CCMLite kernel for Trainium2: GroupNorm(affine=False) + low-rank channel mix.

out = x_norm + u @ (v^T @ x_norm) + shift, with x_norm = groupnorm(x).

Sharding: data-parallel over batch B=16 across 8 cores (2 batch elems/core).

DMA floor is ~24us/core (8.4 MB at ~360 GB/s). DVE/ACT are the only engines
that can read PSUM; latency ladders (stats -> group chain -> vs) gate each
batch, so they are kept short and high-priority:
  - all x loads issue up-front on the sync ring; the last tile of each batch
    is split in half so the gating stats start earlier
  - stats ops are sliced to 1024 columns so chain rungs never wait long
    behind bulk ops; s1 via DVE tensor_reduce (runs in DVE's idle load
    window) or ACT Copy+accum; s2 via ACT Square+accum or DVE STT+accum
  - group chain: 1/GPC is folded into the host-side mask so the gs matmul
    yields group means/E[x^2] directly; Rsqrt fuses sqrt+reciprocal
  - stage A (vtx = vs^T x) packs 4 rank-12 strips per PSUM tile via
    tile_position (they pipeline in distinct PE column groups); lhsT is
    zero-padded to 32 cols so strips cover all 128 partitions; the evac op
    subtracts kvec (mean correction) as a per-partition scalar and maps the
    zero rows 32q+12 to +1 (kvecP=-1 there), creating the ones-row that
    carries cst through stage-B matmuls
  - stage B units of [128,1024]: path D = DVE STT (s*x + pm) from PSUM,
    path A = PE diag(s) matmul + ACT Identity+bias, path E = DVE 4x t=s*x,
    ACT evac, Pool TT-add; per-unit stores stream on the sync ring
"""

from contextlib import ExitStack

import numpy as np

import concourse.bass as bass
import concourse.tile as tile
from concourse import bacc, mybir
from concourse.bass_utils import run_bass_kernel_spmd

N_CORES = 8
B, C, H, W = 16, 256, 64, 64
HW = H * W            # 4096
R = 12                # low rank
G = 32                # groups
GPC = C // G          # 8 channels per group
P = 128               # partitions
CB = C // P           # 2 channel blocks
BPC = B // N_CORES    # 2 batch elements per core
EPS = 1e-6
F32 = mybir.dt.float32
F16 = mybir.dt.float16

_MULT = mybir.AluOpType.mult
_ADD = mybir.AluOpType.add
AF = mybir.ActivationFunctionType

# ---- schedule knobs ----
# s1 (sum of x) per (b, cb): 'red' = DVE tensor_reduce, 'cpy' = ACT Copy+accum
DEF_S1 = {(0, 0): "red", (0, 1): "red", (1, 0): "red", (1, 1): "cpy"}
# s2 per (b, cb): 'act' = ACT Square+accum, 'stt' = DVE STT+accum
DEF_S2 = {(0, 0): "act", (0, 1): "act", (1, 0): "act", (1, 1): "act"}
# stage-B path per batch: 8 chars, unit order u = 2k+cb (k-major)
DEF_PATHS = ("DEDEDEDE", "DDEDDEDE")


def build_nc(paths=DEF_PATHS, s1cfg=DEF_S1, s2cfg=DEF_S2):
    nc = bacc.Bacc(None, target_bir_lowering=False)
    x_d = nc.dram_tensor("x", [BPC, C, HW], F16, kind="ExternalInput")
    aug_d = nc.dram_tensor("aug", [BPC, P, C], F16, kind="ExternalInput")
    vsh_d = nc.dram_tensor("vsh", [BPC, CB, P, 33], F32, kind="ExternalInput")
    gmaskG_d = nc.dram_tensor("gmaskG", [P, 16], F32, kind="ExternalInput")
    gmaskT_d = nc.dram_tensor("gmaskT", [16, P], F32, kind="ExternalInput")
    ident_d = nc.dram_tensor("ident16", [P, P], F16, kind="ExternalInput")
    kinit_d = nc.dram_tensor("kinit", [16, 1], F32, kind="ExternalInput")
    ones_d = nc.dram_tensor("ones16", [1, 4], F16, kind="ExternalInput")
    out_d = nc.dram_tensor("out", [BPC, C, HW], F16, kind="ExternalOutput")

    with tile.TileContext(nc) as tc, ExitStack() as ctx:
        consts = ctx.enter_context(tc.tile_pool(name="consts", bufs=1))
        xbp = ctx.enter_context(tc.tile_pool(name="xbp", bufs=6))
        junkp = ctx.enter_context(tc.tile_pool(name="junkp", bufs=2))
        outp = ctx.enter_context(tc.tile_pool(name="outp", bufs=6))
        vtp = ctx.enter_context(tc.tile_pool(name="vtp", bufs=4))
        smalls = ctx.enter_context(tc.tile_pool(name="smalls", bufs=2))
        ps_small = ctx.enter_context(
            tc.tile_pool(name="ps_small", bufs=2, space="PSUM"))
        ps_vtx = ctx.enter_context(
            tc.tile_pool(name="ps_vtx", bufs=1, space="PSUM"))
        ps_pm = ctx.enter_context(tc.tile_pool(name="ps_pm", bufs=2, space="PSUM"))

        # ---- consts ----
        gmaskG = consts.tile([P, 16], F32)   # group mask * 1/GPC
        nc.gpsimd.dma_start(out=gmaskG, in_=gmaskG_d[:, :])
        gmaskT = consts.tile([16, P], F32)
        nc.gpsimd.dma_start(out=gmaskT, in_=gmaskT_d[:, :])
        ident_h = consts.tile([P, P], F16)
        nc.gpsimd.dma_start(out=ident_h, in_=ident_d[:, :])
        kinit = consts.tile([16, 1], F32)
        nc.gpsimd.dma_start(out=kinit, in_=kinit_d[:, :])
        ones14 = consts.tile([1, 4], F16)
        nc.gpsimd.dma_start(out=ones14, in_=ones_d[:, :])
        eps_t = consts.tile([16, 1], F32)
        nc.vector.memset(eps_t, EPS)

        # per-batch params on the scalar HWDGE ring (issued before x loads)
        augs, vshs = [], []
        for b in range(BPC):
            aug = smalls.tile([P, 2 * P], F16, tag=f"aug{b}", bufs=1)
            nc.scalar.dma_start(out=aug, in_=aug_d[b])
            vsh = smalls.tile([P, 66], F32, tag=f"vsh{b}", bufs=1)
            for cb in range(CB):
                nc.scalar.dma_start(
                    out=vsh[:, 33 * cb:33 * (cb + 1)], in_=vsh_d[b, cb])
            augs.append(aug)
            vshs.append(vsh)

        # warm ACT tables early so Rsqrt/Square don't table-load mid-kernel
        twarm = smalls.tile([16, 1], F32, tag="twarm", bufs=1)
        nc.scalar.activation(out=twarm, in_=eps_t, func=AF.Square)
        nc.scalar.activation(out=twarm, in_=eps_t, func=AF.Sqrt,
                             bias=eps_t[:, 0:1], scale=1.0)
        nc.scalar.activation(out=twarm, in_=eps_t, func=AF.Identity)
        nc.scalar.activation(out=twarm, in_=eps_t, func=AF.Copy)

        # ---- all x loads up-front, sync ring; last tile of each batch split
        # so the tail-gating stats start earlier ----
        # xparts[(b,cb,h)] = list of (tile, col0, width) covering cols 0..2048
        xparts = {}
        for b in range(BPC):
            for cb in range(CB):
                for h in range(2):
                    if cb == 1 and h == 1:
                        parts = []
                        for i in range(2):
                            tb = xbp.tile([P, 1024], F16, tag="xbt2",
                                          bufs=4, name=f"xl{b}{i}")
                            nc.sync.dma_start(
                                out=tb,
                                in_=x_d[b, cb * P:(cb + 1) * P,
                                        h * 2048 + 1024 * i:
                                        h * 2048 + 1024 * (i + 1)])
                            parts.append((tb, 1024 * i, 1024))
                        xparts[(b, cb, h)] = parts
                    else:
                        tb = xbp.tile([P, 2048], F16, tag="xbt",
                                      name=f"x{b}{cb}{h}")
                        nc.sync.dma_start(
                            out=tb,
                            in_=x_d[b, cb * P:(cb + 1) * P,
                                    h * 2048:(h + 1) * 2048])
                        xparts[(b, cb, h)] = [(tb, 0, 2048)]

        def x_ap(b, cb, col0, width):
            h, c = col0 // 2048, col0 % 2048
            for tile_, t0, tw in xparts[(b, cb, h)]:
                if t0 <= c and c + width <= t0 + tw:
                    return tile_[:, c - t0:c - t0 + width]
            raise AssertionError((b, cb, col0, width))

        def x_slices(b, cb, h):
            # 1024-wide (tile, slice) pieces of column range [2048h, 2048h+2048)
            out = []
            for tile_, t0, tw in xparts[(b, cb, h)]:
                for i in range(tw // 1024):
                    out.append(tile_[:, 1024 * i:1024 * (i + 1)])
            return out

        sms = {}     # (b,cb) -> [128,2] f32: col0 rstd, col1 group-mean
        vss = {}     # (b,cb) -> [128,32] f16 (v*s, zero-padded)
        diags = {}   # (b,cb) -> [128,128] f16 diag(s)
        kvsb = {}    # (b,cb) -> [R,1] f32 partial kvec
        kvecPs = {}  # b -> [128,1] f32 strip-replicated kvec (rows 32q+12=-1)
        accs = {}    # (b,cb) -> [128,8] f32: s1 cols 0..3, s2 cols 4..7
        csts = {}
        vts = {}
        vtx_ps = {}

        def get_acc(b, cb):
            if (b, cb) not in accs:
                accs[(b, cb)] = smalls.tile(
                    [P, 8], F32, tag=f"acc{b}{cb}", bufs=1, name=f"acc{b}{cb}")
            return accs[(b, cb)]

        def emit_stats(b, cb, h):
            # sliced s1 + s2 for x cols [2048h, 2048h+2048)
            acc = get_acc(b, cb)
            for i, sl in enumerate(x_slices(b, cb, h)):
                c1, c2 = 2 * h + i, 4 + 2 * h + i
                if s1cfg[(b, cb)] == "red":
                    nc.vector.tensor_reduce(
                        out=acc[:, c1:c1 + 1], in_=sl,
                        axis=mybir.AxisListType.X, op=_ADD)
                else:  # cpy
                    jc = junkp.tile([P, 1024], F16, tag="jc")
                    nc.scalar.activation(
                        out=jc, in_=sl, func=AF.Copy,
                        accum_out=acc[:, c1:c1 + 1])
                if s2cfg[(b, cb)] == "act":
                    ja = junkp.tile([P, 1024], F16, tag="ja")
                    nc.scalar.activation(
                        out=ja, in_=sl, func=AF.Square,
                        accum_out=acc[:, c2:c2 + 1])
                else:
                    jd = junkp.tile([P, 1024], F16, tag="jd")
                    nc.vector.scalar_tensor_tensor(
                        out=jd, in0=sl, scalar=1.0, in1=sl,
                        op0=_MULT, op1=_MULT, accum_out=acc[:, c2:c2 + 1])

        def fold4(acc, c0, out):
            # out = (acc[c0]+acc[c0+1]+acc[c0+2]+acc[c0+3]) / HW
            nc.vector.tensor_scalar(
                out=acc[:, c0:c0 + 1], in0=acc[:, c0:c0 + 1],
                scalar1=acc[:, c0 + 1:c0 + 2], scalar2=acc[:, c0 + 2:c0 + 3],
                op0=_ADD, op1=_ADD)
            nc.vector.tensor_scalar(
                out=out, in0=acc[:, c0:c0 + 1],
                scalar1=acc[:, c0 + 3:c0 + 4], scalar2=1.0 / HW,
                op0=_ADD, op1=_MULT)

        def emit_chain(b, cb):
            # short ladder: msum -> gs(mm) -> var -> Rsqrt -> bc(mm) -> sm -> vs
            acc = accs[(b, cb)]
            msum = smalls.tile([P, 2], F32, tag="msum")
            fold4(acc, 0, msum[:, 0:1])
            fold4(acc, 4, msum[:, 1:2])
            gs = ps_small.tile([16, 2], F32, tag="ps")
            nc.tensor.matmul(gs, lhsT=gmaskG, rhs=msum, start=True, stop=True)
            gvals = smalls.tile([16, 2], F32, tag="gvals")
            tmpg = smalls.tile([16, 2], F32, tag="tmpg")
            nc.vector.tensor_copy(out=gvals[:, 1:2], in_=gs[:, 0:1])
            nc.vector.tensor_mul(out=tmpg[:, 0:1], in0=gvals[:, 1:2],
                                 in1=gvals[:, 1:2])
            nc.vector.tensor_sub(out=tmpg[:, 1:2], in0=gs[:, 1:2],
                                 in1=tmpg[:, 0:1])
            gsd = smalls.tile([16, 1], F32, tag="gsd")
            nc.scalar.activation(
                out=gsd, in_=tmpg[:, 1:2], func=AF.Sqrt,
                bias=eps_t[:, 0:1], scale=1.0)
            nc.vector.reciprocal(out=gvals[:, 0:1], in_=gsd)
            bc = ps_small.tile([P, 2], F32, tag="ps")
            nc.tensor.matmul(bc, lhsT=gmaskT, rhs=gvals, start=True, stop=True)
            sm = smalls.tile([P, 2], F32, tag=f"sm{b}{cb}", bufs=1)
            nc.vector.tensor_copy(out=sm, in_=bc)
            sms[(b, cb)] = sm
            # vs = v * s (fp16, zero-padded cols 12..31 so stage A strips
            # cover all 128 partitions)
            vs = smalls.tile([P, 32], F16, tag=f"vs{b}{cb}", bufs=1)
            nc.vector.tensor_scalar_mul(
                out=vs, in0=vshs[b][:, 33 * cb:33 * cb + 32],
                scalar1=sm[:, 0:1])
            vss[(b, cb)] = vs
            if "A" in paths[b]:
                diag = smalls.tile([P, P], F16, tag=f"diag{b}{cb}", bufs=1)
                nc.vector.tensor_scalar_mul(out=diag, in0=ident_h,
                                            scalar1=sm[:, 0:1])
                diags[(b, cb)] = diag

        def emit_cst(b):
            aug = augs[b]
            for cb in range(CB):
                sm = sms[(b, cb)]
                ms = smalls.tile([P, 1], F32, tag=f"ms{b}{cb}", bufs=1)
                nc.vector.tensor_mul(out=ms, in0=sm[:, 1:2], in1=sm[:, 0:1])
                # kvec partial: kv[r] = sum_c v[c,r] * (m*s)_c
                kv = ps_small.tile([32, 1], F32, tag="ps")
                nc.tensor.matmul(
                    kv, lhsT=vshs[b][:, 33 * cb:33 * cb + 32], rhs=ms,
                    start=True, stop=True)
                kvp = smalls.tile([R, 1], F32, tag=f"kv{b}{cb}", bufs=1)
                nc.vector.tensor_copy(out=kvp, in_=kv[0:R, :])
                kvsb[(b, cb)] = kvp
                # cst = shift - m*s
                cst = smalls.tile([P, 1], F32, tag=f"cst{b}{cb}", bufs=1)
                nc.vector.tensor_sub(
                    out=cst, in0=vshs[b][:, 33 * cb + 32:33 * cb + 33], in1=ms)
                csts[(b, cb)] = cst
                cst16 = smalls.tile([P, 1], F16, tag="cst16")
                nc.vector.tensor_copy(out=cst16, in_=cst)
                ctp = ps_small.tile([1, P], F16, tag="ps")
                nc.tensor.transpose(out=ctp, in_=cst16, identity=ident_h)
                cstrow = smalls.tile([1, P], F16, tag="cstrow")
                nc.scalar.copy(out=cstrow, in_=ctp)
                ctp4 = ps_small.tile([4, P], F32, tag="ps")
                nc.tensor.matmul(ctp4, lhsT=ones14, rhs=cstrow,
                                 start=True, stop=True)
                cstrow4 = smalls.tile([4, P], F16, tag="cstrow4")
                nc.scalar.copy(out=cstrow4, in_=ctp4)
                pstride = aug.ap[0][0]
                dst = bass.AP(
                    tensor=aug.tensor,
                    offset=aug.offset + R * pstride + P * cb,
                    ap=[[32 * pstride, 4], [1, P]])
                nc.gpsimd.dma_start(out=dst, in_=cstrow4)
            # kvecP: rows 32q+r = kvec[r], rows 32q+12 = -1 (evac's 0-kvecP
            # gives the +1 ones-row), rest 0
            krow = smalls.tile([16, 1], F32, tag=f"krow{b}", bufs=1)
            nc.gpsimd.dma_start(out=krow, in_=kinit[:, :])
            nc.vector.tensor_add(
                out=krow[0:R, :], in0=kvsb[(b, 0)], in1=kvsb[(b, 1)])
            kvecP = smalls.tile([P, 1], F32, tag=f"kvecP{b}", bufs=1)
            for q in range(4):
                nc.gpsimd.dma_start(out=kvecP[32 * q:32 * q + 16, :], in_=krow)
            kvecPs[b] = kvecP

        def emit_stage_a(b, ch):
            # vtx strips for chunks j = 4*ch + q; strip q covers partitions
            # 32q..32q+31 (rows 12..31 zero via the padded lhsT)
            if ch == 0:
                vtx_ps[b] = ps_vtx.tile([P, 1024], F32, tag="vtx",
                                        name=f"vtx{b}")
            vps = vtx_ps[b]
            for cb in range(CB):
                for q in range(4):
                    nc.tensor.matmul(
                        vps[32 * q:32 * q + 32, 512 * ch:512 * (ch + 1)],
                        lhsT=vss[(b, cb)],
                        rhs=x_ap(b, cb, 2048 * ch + 512 * q, 512),
                        start=(cb == 0), stop=(cb == CB - 1),
                        tile_position=(0, 32 * q),
                        skip_group_check=True)

        def emit_evac(b, ch):
            # vt = vtx - kvec; zero rows 32q+12 become +1 (kvecP=-1 there)
            vt = vtp.tile([P, 512], F16, tag="vt")
            nc.vector.tensor_scalar_sub(
                out=vt, in0=vtx_ps[b][:, 512 * ch:512 * (ch + 1)],
                scalar1=kvecPs[b])
            vts[(b, ch)] = vt

        def emit_unit(b, k, cb):
            # output unit [128,1024]: chunks (2k, 2k+1); vtx strips
            # q = 2k%4, (2k+1)%4 of vts[(b, k//2)]
            path = paths[b][2 * k + cb]
            aug = augs[b]
            sm = sms[(b, cb)]
            xap = x_ap(b, cb, 1024 * k, 1024)
            pm = ps_pm.tile([P, 1024], F32, tag="pm")
            vt = vts[(b, k // 2)]
            for j2 in range(2):
                q = (2 * k + j2) % 4
                pslice = pm[:, 512 * j2:512 * (j2 + 1)]
                if path == "A":
                    nc.tensor.matmul(
                        pslice, lhsT=diags[(b, cb)],
                        rhs=x_ap(b, cb, 1024 * k + 512 * j2, 512),
                        start=True, stop=False,
                        skip_group_check=True)
                    nc.tensor.matmul(
                        pslice,
                        lhsT=aug[32 * q:32 * q + R, P * cb:P * (cb + 1)],
                        rhs=vt[32 * q:32 * q + R, :],
                        start=False, stop=True,
                        tile_position=(32 * q, 0),
                        skip_group_check=True)
                else:
                    nc.tensor.matmul(
                        pslice,
                        lhsT=aug[32 * q:32 * q + R + 1, P * cb:P * (cb + 1)],
                        rhs=vt[32 * q:32 * q + R + 1, :],
                        start=True, stop=True,
                        tile_position=(32 * q, 0),
                        skip_group_check=True)
            osb = outp.tile([P, 1024], F16, tag="osb")
            if path == "A":
                nc.scalar.activation(
                    out=osb, in_=pm, func=AF.Identity,
                    bias=csts[(b, cb)], scale=1.0)
            elif path == "D":
                nc.vector.scalar_tensor_tensor(
                    out=osb, in0=xap, scalar=sm[:, 0:1], in1=pm,
                    op0=_MULT, op1=_ADD)
            else:  # E
                t = outp.tile([P, 1024], F16, tag="tsx", bufs=3)
                nc.vector.tensor_scalar(
                    out=t, in0=xap, scalar1=sm[:, 0:1], scalar2=0.0,
                    op0=_MULT, op1=_ADD)
                pmsb = outp.tile([P, 1024], F16, tag="pmsb", bufs=3)
                nc.scalar.activation(out=pmsb, in_=pm, func=AF.Identity)
                nc.gpsimd.tensor_add(out=osb, in0=t, in1=pmsb)
            nc.sync.dma_start(
                out=out_d[b, cb * P:(cb + 1) * P, 1024 * k:1024 * (k + 1)],
                in_=osb)

        # ================= schedule =================
        for cb in range(CB):
            emit_stats(0, cb, 0)
            emit_stats(0, cb, 1)
            emit_chain(0, cb)
        emit_cst(0)
        emit_stats(1, 0, 0)
        emit_stats(1, 0, 1)
        emit_stage_a(0, 0)
        emit_evac(0, 0)
        emit_stats(1, 1, 0)
        emit_stats(1, 1, 1)
        emit_stage_a(0, 1)
        emit_evac(0, 1)
        emit_chain(1, 0)
        emit_chain(1, 1)
        emit_cst(1)
        for k in range(4):
            for cb in range(CB):
                emit_unit(0, k, cb)
        emit_stage_a(1, 0)
        emit_evac(1, 0)
        emit_stage_a(1, 1)
        emit_evac(1, 1)
        for k in range(4):
            for cb in range(CB):
                emit_unit(1, k, cb)

    nc.finalize()
    return nc


def _host_prep(x, ccm_params):
    x = np.asarray(x, dtype=np.float32).reshape(B, C, HW).astype(np.float16)
    x = np.ascontiguousarray(x)
    cp = np.asarray(ccm_params, dtype=np.float32)
    u = cp[:, :C * R].reshape(B, C, R)
    v = cp[:, C * R:2 * C * R].reshape(B, C, R)
    shift = cp[:, 2 * C * R:].reshape(B, C)
    # aug: [B, 128, C] fp16; strips s=0..3: rows 32s..32s+11 = u^T,
    # row 32s+12 = cst written on device
    aug = np.zeros((B, P, C), np.float16)
    ut = u.transpose(0, 2, 1).astype(np.float16)
    for sx in range(4):
        aug[:, 32 * sx:32 * sx + R, :] = ut
    aug = np.ascontiguousarray(aug)
    # vsh: [B, CB, P, 33] f32: cols 0..11 = v, 12..31 zero pad, col 32 = shift
    vsh = np.zeros((B, CB, P, 33), np.float32)
    vsh[..., :R] = v.reshape(B, CB, P, R)
    vsh[..., 32] = shift.reshape(B, CB, P)
    vsh = np.ascontiguousarray(vsh)
    gmask = np.zeros((P, 16), np.float32)
    gmask[np.arange(P), np.arange(P) // GPC] = 1.0
    gmaskG = np.ascontiguousarray(gmask / GPC)
    gmaskT = np.ascontiguousarray(gmask.T)
    ident16 = np.eye(P, dtype=np.float16)
    kinit = np.zeros((16, 1), np.float32)
    kinit[12, 0] = -1.0
    in_maps = []
    for c in range(N_CORES):
        bs = slice(c * BPC, (c + 1) * BPC)
        in_maps.append({
            "x": x[bs], "aug": aug[bs], "vsh": vsh[bs],
            "gmaskG": gmaskG, "gmaskT": gmaskT, "ident16": ident16,
            "kinit": kinit, "ones16": np.ones((1, 4), np.float16),
        })
    return in_maps


def kernel(x, ccm_params, _trace=False, _paths=DEF_PATHS, _s1=DEF_S1,
           _s2=DEF_S2, **_ignored):
    in_maps = _host_prep(x, ccm_params)
    nc = build_nc(paths=_paths, s1cfg=_s1, s2cfg=_s2)
    res = run_bass_kernel_spmd(
        nc, in_maps, core_ids=list(range(N_CORES)), trace=_trace)
    out = np.concatenate([r["out"] for r in res.results], axis=0)
    out = out.reshape(B, C, H, W).astype(np.float32, copy=False)
    if _trace:
        return out, res
    return out


# revision 18
# speedup vs baseline: 1.2661x; 1.0114x over previous
"""CCMLite kernel for Trainium2: GroupNorm(affine=False) + low-rank channel mix.

out = x_norm + u @ (v^T @ x_norm) + shift, with x_norm = groupnorm(x).

Sharding: data-parallel over batch B=16 across 8 cores (2 batch elems/core).

DMA floor is ~24us/core (8.4 MB at ~360 GB/s). DVE/ACT are the only engines
that can read PSUM; latency ladders (stats -> group chain -> vs) gate each
batch, so they are kept short and high-priority:
  - all x loads issue up-front on the sync ring; the last tile of each batch
    is split in half so the gating stats start earlier
  - stats ops are sliced to 1024 columns so chain rungs never wait long
    behind bulk ops; s1 via DVE tensor_reduce (runs in DVE's idle load
    window) or ACT Copy+accum; s2 via ACT Square+accum or DVE STT+accum
  - group chain: 1/GPC is folded into the host-side mask so the gs matmul
    yields group means/E[x^2] directly; Rsqrt fuses sqrt+reciprocal
  - stage A (vtx = vs^T x) packs 4 rank-12 strips per PSUM tile via
    tile_position (they pipeline in distinct PE column groups); lhsT is
    zero-padded to 32 cols so strips cover all 128 partitions; the evac op
    subtracts kvec (mean correction) as a per-partition scalar and maps the
    zero rows 32q+12 to +1 (kvecP=-1 there), creating the ones-row that
    carries cst through stage-B matmuls
  - stage B units of [128,1024]: path D = DVE STT (s*x + pm) from PSUM,
    path A = PE diag(s) matmul + ACT Identity+bias, path E = DVE 4x t=s*x,
    ACT evac, Pool TT-add; per-unit stores stream on the sync ring
"""

from contextlib import ExitStack

import numpy as np

import concourse.bass as bass
import concourse.tile as tile
from concourse import bacc, mybir
from concourse.bass_utils import run_bass_kernel_spmd

N_CORES = 8
B, C, H, W = 16, 256, 64, 64
HW = H * W            # 4096
R = 12                # low rank
G = 32                # groups
GPC = C // G          # 8 channels per group
P = 128               # partitions
CB = C // P           # 2 channel blocks
BPC = B // N_CORES    # 2 batch elements per core
EPS = 1e-6
F32 = mybir.dt.float32
F16 = mybir.dt.float16

_MULT = mybir.AluOpType.mult
_ADD = mybir.AluOpType.add
AF = mybir.ActivationFunctionType

# ---- schedule knobs ----
# s2 per (b, cb): 'act' = ACT Square+accum, 'stt' = DVE STT+accum
# (s1 always rides tile_position-packed PE group-sum matmuls)
DEF_S2 = {(0, 0): "act", (0, 1): "stt", (1, 0): "act", (1, 1): "stt"}
# stage-B path per batch: 8 chars, unit order u = 2k+cb (k-major)
DEF_PATHS = ("DEADEDAE", "DAEDEDDE")


def build_nc(paths=DEF_PATHS, s2cfg=DEF_S2):
    nc = bacc.Bacc(None, target_bir_lowering=False)
    x_d = nc.dram_tensor("x", [BPC, C, HW], F16, kind="ExternalInput")
    aug_d = nc.dram_tensor("aug", [BPC, P, C], F16, kind="ExternalInput")
    vsh_d = nc.dram_tensor("vsh", [BPC, CB, P, 33], F32, kind="ExternalInput")
    gmaskG_d = nc.dram_tensor("gmaskG", [P, 16], F32, kind="ExternalInput")
    gmask16_d = nc.dram_tensor("gmask16", [P, 32], F16, kind="ExternalInput")
    foldm_d = nc.dram_tensor("foldm", [P, 16], F32, kind="ExternalInput")
    gmaskT_d = nc.dram_tensor("gmaskT", [16, P], F32, kind="ExternalInput")
    ident_d = nc.dram_tensor("ident16", [P, P], F16, kind="ExternalInput")
    kinit_d = nc.dram_tensor("kinit", [16, 1], F32, kind="ExternalInput")
    ones_d = nc.dram_tensor("ones16", [1, 4], F16, kind="ExternalInput")
    out_d = nc.dram_tensor("out", [BPC, C, HW], F16, kind="ExternalOutput")

    with tile.TileContext(nc) as tc, ExitStack() as ctx:
        consts = ctx.enter_context(tc.tile_pool(name="consts", bufs=1))
        xbp = ctx.enter_context(tc.tile_pool(name="xbp", bufs=6))
        junkp = ctx.enter_context(tc.tile_pool(name="junkp", bufs=2))
        outp = ctx.enter_context(tc.tile_pool(name="outp", bufs=6))
        vtp = ctx.enter_context(tc.tile_pool(name="vtp", bufs=4))
        smalls = ctx.enter_context(tc.tile_pool(name="smalls", bufs=2))
        ps_small = ctx.enter_context(
            tc.tile_pool(name="ps_small", bufs=2, space="PSUM"))
        ps_vtx = ctx.enter_context(
            tc.tile_pool(name="ps_vtx", bufs=1, space="PSUM"))
        ps_pm = ctx.enter_context(tc.tile_pool(name="ps_pm", bufs=2, space="PSUM"))

        # ---- consts ----
        gmaskG = consts.tile([P, 16], F32)   # group mask * 1/GPC
        nc.gpsimd.dma_start(out=gmaskG, in_=gmaskG_d[:, :])
        gmask16 = consts.tile([P, 32], F16)  # group mask, zero-padded
        nc.gpsimd.dma_start(out=gmask16, in_=gmask16_d[:, :])
        foldm = consts.tile([P, 16], F32)    # strip fold mask / (GPC*HW)
        nc.gpsimd.dma_start(out=foldm, in_=foldm_d[:, :])
        gmaskT = consts.tile([16, P], F32)
        nc.gpsimd.dma_start(out=gmaskT, in_=gmaskT_d[:, :])
        ident_h = consts.tile([P, P], F16)
        nc.gpsimd.dma_start(out=ident_h, in_=ident_d[:, :])
        kinit = consts.tile([16, 1], F32)
        nc.gpsimd.dma_start(out=kinit, in_=kinit_d[:, :])
        ones14 = consts.tile([1, 4], F16)
        nc.gpsimd.dma_start(out=ones14, in_=ones_d[:, :])
        eps_t = consts.tile([16, 1], F32)
        nc.vector.memset(eps_t, EPS)

        # per-batch params on the scalar HWDGE ring (issued before x loads)
        augs, vshs = [], []
        for b in range(BPC):
            aug = smalls.tile([P, 2 * P], F16, tag=f"aug{b}", bufs=1)
            nc.scalar.dma_start(out=aug, in_=aug_d[b])
            vsh = smalls.tile([P, 66], F32, tag=f"vsh{b}", bufs=1)
            for cb in range(CB):
                nc.scalar.dma_start(
                    out=vsh[:, 33 * cb:33 * (cb + 1)], in_=vsh_d[b, cb])
            augs.append(aug)
            vshs.append(vsh)

        # warm ACT tables early so Rsqrt/Square don't table-load mid-kernel
        twarm = smalls.tile([16, 1], F32, tag="twarm", bufs=1)
        nc.scalar.activation(out=twarm, in_=eps_t, func=AF.Square)
        nc.scalar.activation(out=twarm, in_=eps_t, func=AF.Sqrt,
                             bias=eps_t[:, 0:1], scale=1.0)
        nc.scalar.activation(out=twarm, in_=eps_t, func=AF.Identity)
        nc.scalar.activation(out=twarm, in_=eps_t, func=AF.Copy)

        # ---- all x loads up-front, sync ring; last tile of each batch split
        # so the tail-gating stats start earlier ----
        # xparts[(b,cb,h)] = list of (tile, col0, width) covering cols 0..2048
        xparts = {}
        for b in range(BPC):
            for cb in range(CB):
                for h in range(2):
                    if cb == 1 and h == 1:
                        parts = []
                        for i in range(2):
                            tb = xbp.tile([P, 1024], F16, tag="xbt2",
                                          bufs=4, name=f"xl{b}{i}")
                            nc.sync.dma_start(
                                out=tb,
                                in_=x_d[b, cb * P:(cb + 1) * P,
                                        h * 2048 + 1024 * i:
                                        h * 2048 + 1024 * (i + 1)])
                            parts.append((tb, 1024 * i, 1024))
                        xparts[(b, cb, h)] = parts
                    else:
                        tb = xbp.tile([P, 2048], F16, tag="xbt",
                                      name=f"x{b}{cb}{h}")
                        nc.sync.dma_start(
                            out=tb,
                            in_=x_d[b, cb * P:(cb + 1) * P,
                                    h * 2048:(h + 1) * 2048])
                        xparts[(b, cb, h)] = [(tb, 0, 2048)]

        def x_ap(b, cb, col0, width):
            h, c = col0 // 2048, col0 % 2048
            for tile_, t0, tw in xparts[(b, cb, h)]:
                if t0 <= c and c + width <= t0 + tw:
                    return tile_[:, c - t0:c - t0 + width]
            raise AssertionError((b, cb, col0, width))

        def x_slices(b, cb, h):
            # 1024-wide (tile, slice) pieces of column range [2048h, 2048h+2048)
            out = []
            for tile_, t0, tw in xparts[(b, cb, h)]:
                for i in range(tw // 1024):
                    out.append(tile_[:, 1024 * i:1024 * (i + 1)])
            return out

        sms = {}     # (b,cb) -> [128,2] f32: col0 rstd, col1 group-mean
        vss = {}     # (b,cb) -> [128,32] f16 (v*s, zero-padded)
        diags = {}   # (b,cb) -> [128,128] f16 diag(s)
        kvsb = {}    # (b,cb) -> [R,1] f32 partial kvec
        kvecPs = {}  # b -> [128,1] f32 strip-replicated kvec (rows 32q+12=-1)
        accs = {}    # (b,cb) -> [128,4] f32 s2 accumulator columns
        naccs = {}   # (b,cb) -> number of s2 cols used
        gxs = {}     # (b,cb) -> [128,512] PSUM packed group partial sums
        csts = {}
        vts = {}
        vtx_ps = {}

        def get_acc(b, cb):
            if (b, cb) not in accs:
                accs[(b, cb)] = smalls.tile(
                    [P, 4], F32, tag=f"acc{b}{cb}", bufs=1, name=f"acc{b}{cb}")
                naccs[(b, cb)] = 0
            return accs[(b, cb)]

        def emit_stats(b, cb, h):
            # s1: packed group-sum matmuls -- strip q of the shared [128,512]
            # PSUM tile accumulates chunks q and q+4 at PE column 32q
            if h == 0:
                gxs[(b, cb)] = ps_small.tile([P, 512], F32, tag="ps",
                                             name=f"gx{b}{cb}")
            gx = gxs[(b, cb)]
            for q in range(4):
                nc.tensor.matmul(
                    gx[32 * q:32 * q + 32, :], lhsT=gmask16,
                    rhs=x_ap(b, cb, 2048 * h + 512 * q, 512),
                    start=(h == 0), stop=(h == 1),
                    tile_position=(0, 32 * q),
                    skip_group_check=True)
            # s2 per natural tile piece
            acc = get_acc(b, cb)
            for tile_, t0, tw in xparts[(b, cb, h)]:
                col = naccs[(b, cb)]
                naccs[(b, cb)] += 1
                if s2cfg[(b, cb)] == "act":
                    ja = junkp.tile([P, 2048], F16, tag="ja")
                    nc.scalar.activation(
                        out=ja[:, 0:tw], in_=tile_, func=AF.Square,
                        accum_out=acc[:, col:col + 1])
                else:
                    jd = junkp.tile([P, 2048], F16, tag="jd")
                    nc.vector.scalar_tensor_tensor(
                        out=jd[:, 0:tw], in0=tile_, scalar=1.0, in1=tile_,
                        op0=_MULT, op1=_MULT, accum_out=acc[:, col:col + 1])

        def fold(acc, n, out):
            # out = sum(acc[:, 0:n]) / HW
            if n == 2:
                nc.vector.tensor_scalar(
                    out=out, in0=acc[:, 0:1], scalar1=acc[:, 1:2],
                    scalar2=1.0 / HW, op0=_ADD, op1=_MULT)
            else:
                assert n == 3
                nc.vector.tensor_scalar(
                    out=acc[:, 0:1], in0=acc[:, 0:1], scalar1=acc[:, 1:2],
                    scalar2=acc[:, 2:3], op0=_ADD, op1=_ADD)
                nc.vector.tensor_scalar_mul(
                    out=out, in0=acc[:, 0:1], scalar1=1.0 / HW)

        gxrs = {}

        def emit_gxred(b, cb):
            # consume the gx PSUM tile early so the ps ring can rotate
            gxr = smalls.tile([P, 1], F32, tag="gxr", bufs=4,
                              name=f"gxr{b}{cb}")
            nc.vector.tensor_reduce(
                out=gxr, in_=gxs[(b, cb)], axis=mybir.AxisListType.X, op=_ADD)
            gxrs[(b, cb)] = gxr

        def emit_chain(b, cb):
            # ladder: mg(mm) / e2 fold -> gs(mm) -> var -> sqrt ->
            # recip -> bc(mm) -> sm -> vs
            if (b, cb) not in gxrs:
                emit_gxred(b, cb)
            mg = ps_small.tile([16, 1], F32, tag="ps")
            nc.tensor.matmul(mg, lhsT=foldm, rhs=gxrs[(b, cb)],
                             start=True, stop=True)
            acc = accs[(b, cb)]
            msum = smalls.tile([P, 1], F32, tag="msum")
            fold(acc, naccs[(b, cb)], msum)
            gs = ps_small.tile([16, 1], F32, tag="ps")
            nc.tensor.matmul(gs, lhsT=gmaskG, rhs=msum, start=True, stop=True)
            gvals = smalls.tile([16, 2], F32, tag="gvals")
            tmpg = smalls.tile([16, 2], F32, tag="tmpg")
            nc.vector.tensor_copy(out=gvals[:, 1:2], in_=mg)
            nc.vector.tensor_mul(out=tmpg[:, 0:1], in0=gvals[:, 1:2],
                                 in1=gvals[:, 1:2])
            nc.vector.tensor_sub(out=tmpg[:, 1:2], in0=gs,
                                 in1=tmpg[:, 0:1])
            gsd = smalls.tile([16, 1], F32, tag="gsd")
            nc.scalar.activation(
                out=gsd, in_=tmpg[:, 1:2], func=AF.Sqrt,
                bias=eps_t[:, 0:1], scale=1.0)
            nc.vector.reciprocal(out=gvals[:, 0:1], in_=gsd)
            bc = ps_small.tile([P, 2], F32, tag="ps")
            nc.tensor.matmul(bc, lhsT=gmaskT, rhs=gvals, start=True, stop=True)
            sm = smalls.tile([P, 2], F32, tag=f"sm{b}{cb}", bufs=1)
            nc.vector.tensor_copy(out=sm, in_=bc)
            sms[(b, cb)] = sm
            # vs = v * s (fp16, zero-padded cols 12..31 so stage A strips
            # cover all 128 partitions)
            vs = smalls.tile([P, 32], F16, tag=f"vs{b}{cb}", bufs=1)
            nc.vector.tensor_scalar_mul(
                out=vs, in0=vshs[b][:, 33 * cb:33 * cb + 32],
                scalar1=sm[:, 0:1])
            vss[(b, cb)] = vs
            if "A" in paths[b]:
                diag = smalls.tile([P, P], F16, tag=f"diag{b}{cb}", bufs=1)
                nc.vector.tensor_scalar_mul(out=diag, in0=ident_h,
                                            scalar1=sm[:, 0:1])
                diags[(b, cb)] = diag

        def emit_cst(b):
            aug = augs[b]
            for cb in range(CB):
                sm = sms[(b, cb)]
                ms = smalls.tile([P, 1], F32, tag=f"ms{b}{cb}", bufs=1)
                nc.vector.tensor_mul(out=ms, in0=sm[:, 1:2], in1=sm[:, 0:1])
                # kvec partial: kv[r] = sum_c v[c,r] * (m*s)_c
                kv = ps_small.tile([32, 1], F32, tag="ps")
                nc.tensor.matmul(
                    kv, lhsT=vshs[b][:, 33 * cb:33 * cb + 32], rhs=ms,
                    start=True, stop=True)
                kvp = smalls.tile([R, 1], F32, tag=f"kv{b}{cb}", bufs=1)
                nc.vector.tensor_copy(out=kvp, in_=kv[0:R, :])
                kvsb[(b, cb)] = kvp
                # cst = shift - m*s
                cst = smalls.tile([P, 1], F32, tag=f"cst{b}{cb}", bufs=1)
                nc.vector.tensor_sub(
                    out=cst, in0=vshs[b][:, 33 * cb + 32:33 * cb + 33], in1=ms)
                csts[(b, cb)] = cst
                cst16 = smalls.tile([P, 1], F16, tag="cst16")
                nc.vector.tensor_copy(out=cst16, in_=cst)
                ctp = ps_small.tile([1, P], F16, tag="ps")
                nc.tensor.transpose(out=ctp, in_=cst16, identity=ident_h)
                cstrow = smalls.tile([1, P], F16, tag="cstrow")
                nc.scalar.copy(out=cstrow, in_=ctp)
                ctp4 = ps_small.tile([4, P], F32, tag="ps")
                nc.tensor.matmul(ctp4, lhsT=ones14, rhs=cstrow,
                                 start=True, stop=True)
                cstrow4 = smalls.tile([4, P], F16, tag="cstrow4")
                nc.scalar.copy(out=cstrow4, in_=ctp4)
                pstride = aug.ap[0][0]
                dst = bass.AP(
                    tensor=aug.tensor,
                    offset=aug.offset + R * pstride + P * cb,
                    ap=[[32 * pstride, 4], [1, P]])
                nc.gpsimd.dma_start(out=dst, in_=cstrow4)
            # kvecP: rows 32q+r = kvec[r], rows 32q+12 = -1 (evac's 0-kvecP
            # gives the +1 ones-row), rest 0
            krow = smalls.tile([16, 1], F32, tag=f"krow{b}", bufs=1)
            nc.gpsimd.dma_start(out=krow, in_=kinit[:, :])
            nc.vector.tensor_add(
                out=krow[0:R, :], in0=kvsb[(b, 0)], in1=kvsb[(b, 1)])
            kvecP = smalls.tile([P, 1], F32, tag=f"kvecP{b}", bufs=1)
            for q in range(4):
                nc.gpsimd.dma_start(out=kvecP[32 * q:32 * q + 16, :], in_=krow)
            kvecPs[b] = kvecP

        def emit_stage_a(b, ch):
            # vtx strips for chunks j = 4*ch + q; strip q covers partitions
            # 32q..32q+31 (rows 12..31 zero via the padded lhsT)
            if ch == 0:
                vtx_ps[b] = ps_vtx.tile([P, 1024], F32, tag="vtx",
                                        name=f"vtx{b}")
            vps = vtx_ps[b]
            for cb in range(CB):
                for q in range(4):
                    nc.tensor.matmul(
                        vps[32 * q:32 * q + 32, 512 * ch:512 * (ch + 1)],
                        lhsT=vss[(b, cb)],
                        rhs=x_ap(b, cb, 2048 * ch + 512 * q, 512),
                        start=(cb == 0), stop=(cb == CB - 1),
                        tile_position=(0, 32 * q),
                        skip_group_check=True)

        def emit_evac(b, ch):
            # vt = vtx - kvec; zero rows 32q+12 become +1 (kvecP=-1 there)
            vt = vtp.tile([P, 512], F16, tag="vt")
            nc.vector.tensor_scalar_sub(
                out=vt, in0=vtx_ps[b][:, 512 * ch:512 * (ch + 1)],
                scalar1=kvecPs[b])
            vts[(b, ch)] = vt

        def emit_unit(b, k, cb):
            # output unit [128,1024]: chunks (2k, 2k+1); vtx strips
            # q = 2k%4, (2k+1)%4 of vts[(b, k//2)]
            path = paths[b][2 * k + cb]
            aug = augs[b]
            sm = sms[(b, cb)]
            xap = x_ap(b, cb, 1024 * k, 1024)
            pm = ps_pm.tile([P, 1024], F32, tag="pm")
            vt = vts[(b, k // 2)]
            for j2 in range(2):
                q = (2 * k + j2) % 4
                pslice = pm[:, 512 * j2:512 * (j2 + 1)]
                if path == "A":
                    nc.tensor.matmul(
                        pslice, lhsT=diags[(b, cb)],
                        rhs=x_ap(b, cb, 1024 * k + 512 * j2, 512),
                        start=True, stop=False,
                        skip_group_check=True)
                    nc.tensor.matmul(
                        pslice,
                        lhsT=aug[32 * q:32 * q + R, P * cb:P * (cb + 1)],
                        rhs=vt[32 * q:32 * q + R, :],
                        start=False, stop=True,
                        tile_position=(32 * q, 0),
                        skip_group_check=True)
                else:
                    nc.tensor.matmul(
                        pslice,
                        lhsT=aug[32 * q:32 * q + R + 1, P * cb:P * (cb + 1)],
                        rhs=vt[32 * q:32 * q + R + 1, :],
                        start=True, stop=True,
                        tile_position=(32 * q, 0),
                        skip_group_check=True)
            osb = outp.tile([P, 1024], F16, tag="osb")
            if path == "A":
                nc.scalar.activation(
                    out=osb, in_=pm, func=AF.Identity,
                    bias=csts[(b, cb)], scale=1.0)
            elif path == "D":
                nc.vector.scalar_tensor_tensor(
                    out=osb, in0=xap, scalar=sm[:, 0:1], in1=pm,
                    op0=_MULT, op1=_ADD)
            else:  # E
                t = outp.tile([P, 1024], F16, tag="tsx", bufs=3)
                nc.vector.tensor_scalar(
                    out=t, in0=xap, scalar1=sm[:, 0:1], scalar2=0.0,
                    op0=_MULT, op1=_ADD)
                pmsb = outp.tile([P, 1024], F16, tag="pmsb", bufs=3)
                nc.scalar.activation(out=pmsb, in_=pm, func=AF.Identity)
                nc.gpsimd.tensor_add(out=osb, in0=t, in1=pmsb)
            nc.sync.dma_start(
                out=out_d[b, cb * P:(cb + 1) * P, 1024 * k:1024 * (k + 1)],
                in_=osb)

        # ================= schedule =================
        for cb in range(CB):
            emit_stats(0, cb, 0)
            emit_stats(0, cb, 1)
            emit_chain(0, cb)
        emit_cst(0)
        emit_stats(1, 0, 0)
        emit_stats(1, 0, 1)
        emit_stage_a(0, 0)
        emit_evac(0, 0)
        emit_gxred(1, 0)
        emit_stats(1, 1, 0)
        emit_stats(1, 1, 1)
        emit_gxred(1, 1)
        emit_stage_a(0, 1)
        emit_evac(0, 1)
        emit_chain(1, 0)
        emit_chain(1, 1)
        emit_cst(1)
        for k in range(4):
            for cb in range(CB):
                emit_unit(0, k, cb)
        emit_stage_a(1, 0)
        emit_evac(1, 0)
        emit_stage_a(1, 1)
        emit_evac(1, 1)
        for k in range(4):
            for cb in range(CB):
                emit_unit(1, k, cb)

    nc.finalize()
    return nc


def _host_prep(x, ccm_params):
    x = np.asarray(x, dtype=np.float32).reshape(B, C, HW).astype(np.float16)
    x = np.ascontiguousarray(x)
    cp = np.asarray(ccm_params, dtype=np.float32)
    u = cp[:, :C * R].reshape(B, C, R)
    v = cp[:, C * R:2 * C * R].reshape(B, C, R)
    shift = cp[:, 2 * C * R:].reshape(B, C)
    # aug: [B, 128, C] fp16; strips s=0..3: rows 32s..32s+11 = u^T,
    # row 32s+12 = cst written on device
    aug = np.zeros((B, P, C), np.float16)
    ut = u.transpose(0, 2, 1).astype(np.float16)
    for sx in range(4):
        aug[:, 32 * sx:32 * sx + R, :] = ut
    aug = np.ascontiguousarray(aug)
    # vsh: [B, CB, P, 33] f32: cols 0..11 = v, 12..31 zero pad, col 32 = shift
    vsh = np.zeros((B, CB, P, 33), np.float32)
    vsh[..., :R] = v.reshape(B, CB, P, R)
    vsh[..., 32] = shift.reshape(B, CB, P)
    vsh = np.ascontiguousarray(vsh)
    gmask = np.zeros((P, 16), np.float32)
    gmask[np.arange(P), np.arange(P) // GPC] = 1.0
    gmaskG = np.ascontiguousarray(gmask / GPC)
    gmaskT = np.ascontiguousarray(gmask.T)
    gmask16 = np.zeros((P, 32), np.float16)
    gmask16[:, :16] = gmask
    foldm = np.zeros((P, 16), np.float32)
    for q in range(4):
        foldm[32 * q + np.arange(16), np.arange(16)] = 1.0 / (GPC * HW)
    foldm = np.ascontiguousarray(foldm)
    ident16 = np.eye(P, dtype=np.float16)
    kinit = np.zeros((16, 1), np.float32)
    kinit[12, 0] = -1.0
    in_maps = []
    for c in range(N_CORES):
        bs = slice(c * BPC, (c + 1) * BPC)
        in_maps.append({
            "x": x[bs], "aug": aug[bs], "vsh": vsh[bs],
            "gmaskG": gmaskG, "gmask16": gmask16, "foldm": foldm,
            "gmaskT": gmaskT, "ident16": ident16,
            "kinit": kinit, "ones16": np.ones((1, 4), np.float16),
        })
    return in_maps


def kernel(x, ccm_params, _trace=False, _paths=DEF_PATHS, _s2=DEF_S2,
           **_ignored):
    in_maps = _host_prep(x, ccm_params)
    nc = build_nc(paths=_paths, s2cfg=_s2)
    res = run_bass_kernel_spmd(
        nc, in_maps, core_ids=list(range(N_CORES)), trace=_trace)
    out = np.concatenate([r["out"] for r in res.results], axis=0)
    out = out.reshape(B, C, H, W).astype(np.float32, copy=False)
    if _trace:
        return out, res
    return out
